# revision 1
# baseline (speedup 1.0000x reference)
"""Trainium2 Bass kernel for EnhancedMetaWeightNetwork.

Full (unsharded) inputs in, full output out. Internally: 8 NeuronCores,
core c handles batch b = c // 2 and query-row half c % 2 (1024 rows).
Attention K/V are computed per-core for the full sequence of the core's
batch (duplicated across the 2 cores sharing a batch; no collectives).

Layout strategy on each core (fp32 storage, fp32r matmuls):
  - activations kept feature-major ("T" = [feature, token]) for the
    attention/projection chain, token-major for the meta-MLP layernorms.
  - scoresT [key, query] per head; softmax denominator via ones-matmul
    restream on the PE; reciprocal broadcast via rank-1 ones matmul.
  - importance lookup via indirect DMA gather from the vocab table.
  - K^T, Q^T, ctx^T spilled through DRAM scratch to fit SBUF.
"""

import numpy as np

H = 1024
NH = 8
HD = 128           # head dim
S = 2048           # keys / full sequence
SQ = 1024          # own query rows per core
MD = 256           # meta dim
MD2 = 128
VOCAB = 32000
MIN_W, MAX_W = 0.1, 5.0
LN_EPS = 1e-5
P = 128
INV_SQRT_HD = 1.0 / np.sqrt(np.float32(HD))

_CACHE = {}


def _build(stop=None):
    """stop in {None, "x", "qkv", "att", "o"}: truncate after that phase
    (debug bisection; a dummy zero output is written instead)."""
    import concourse.bass as bass
    import concourse.mybir as mybir
    import concourse.tile as tile
    from concourse import bacc
    from concourse.masks import make_identity

    f32 = mybir.dt.float32
    f32r = mybir.dt.float32r
    i32 = mybir.dt.int32
    OP = mybir.AluOpType
    ACT = mybir.ActivationFunctionType

    order = {"x": 0, "qkv": 1, "att": 2, "o": 3, "m1": 4, "m2": 5, None: 9}
    lvl = order[stop]

    nc = bacc.Bacc("TRN2", target_bir_lowering=False, debug=False,
                   enable_asserts=False, num_devices=8)

    # ---------------- DRAM parameters ----------------
    dp = nc.declare_dram_parameter
    hT = dp("hT", [H, S], f32, isOutput=False)            # hidden[b].T (own half first)
    pT = dp("pT", [H, S], f32, isOutput=False)            # pos_embed[0].T (own half first)
    wqT = dp("wqT", [H, H], f32r, isOutput=False)         # in_proj_w[0:H].T
    wkT = dp("wkT", [H, H], f32r, isOutput=False)
    wvT = dp("wvT", [H, H], f32r, isOutput=False)
    bq_c = dp("bq_c", [P, H // P], f32, isOutput=False)   # bias, partition-major
    bk_c = dp("bk_c", [P, H // P], f32, isOutput=False)
    bv_b = dp("bv_b", [P, H], f32, isOutput=False)        # bias bcast over partitions
    owT = dp("owT", [H, H], f32r, isOutput=False)         # out_w.T
    ob_c = dp("ob_c", [P, H // P], f32, isOutput=False)
    w1T = dp("w1T", [2 * H, MD], f32r, isOutput=False)
    b1_cd = dp("b1_cd", [P, MD // P], f32, isOutput=False)
    g1_cd = dp("g1_cd", [P, MD // P], f32, isOutput=False)
    be1_cd = dp("be1_cd", [P, MD // P], f32, isOutput=False)
    w2T = dp("w2T", [MD, MD2], f32r, isOutput=False)
    b2_b = dp("b2_b", [P, MD2], f32, isOutput=False)
    g2_b = dp("g2_b", [P, MD2], f32, isOutput=False)
    be2_b = dp("be2_b", [P, MD2], f32, isOutput=False)
    w3_b = dp("w3_b", [P, MD2], f32, isOutput=False)
    b3_c = dp("b3_c", [P, 1], f32, isOutput=False)
    kbias = dp("kbias", [P, S // P], f32, isOutput=False)  # attn mask bias per key
    maskf = dp("maskf", [P, SQ // P], f32, isOutput=False)
    tok = dp("tok", [SQ, 1], i32, isOutput=False)
    table = dp("table", [VOCAB, 1], f32, isOutput=False)
    out = dp("out", [SQ], f32, isOutput=True)

    NKT = S // P          # 16 key tiles
    NC8 = H // P          # 8 feature chunks
    NTT = SQ // P         # 8 own token tiles

    # DRAM scratch
    ktd = nc.dram_tensor("ktd", [H, S], f32r)      # K^T spill
    qtd = nc.dram_tensor("qtd", [H, SQ], f32r)     # Q^T spill
    ctd = nc.dram_tensor("ctd", [H, SQ], f32r)     # ctx^T spill
    atd = nc.dram_tensor("atd", [H, SQ], f32r)     # attended^T spill

    with tile.TileContext(nc) as tc:
        with tc.tile_pool(name="const", bufs=1) as cst, \
             tc.tile_pool(name="xown", bufs=1) as xop:

            # ---------------- constants ----------------
            ones_f = cst.tile([P, P], f32, tag="ones_f")
            nc.any.memset(ones_f[:], 1.0)
            ones_r = cst.tile([P, P], f32r, tag="ones_r")
            nc.vector.tensor_copy(ones_r[:], ones_f[:])
            ident = cst.tile([P, P], f32, tag="ident")
            make_identity(nc, ident[:])
            eps_sb = cst.tile([P, 1], f32, tag="eps")
            nc.any.memset(eps_sb[:], LN_EPS)

            def cload(shape, tag, src):
                t = cst.tile(shape, f32, tag=tag)
                nc.sync.dma_start(t[:], src[:])
                return t

            kbias_sb = cload([P, NKT], "kbias", kbias)
            maskf_sb = cload([P, NTT], "maskf", maskf)
            b3_sb = cload([P, 1], "b3", b3_c)
            w3_sb = cload([P, MD2], "w3", w3_b)
            bq_sb = cload([P, NC8], "bq", bq_c)
            bk_sb = cload([P, NC8], "bk", bk_c)
            ob_sb = cload([P, NC8], "ob", ob_c)
            bv_sb = cload([P, H], "bv", bv_b)
            b1_c = cload([P, MD // P], "b1c", b1_cd)
            g1_c = cload([P, MD // P], "g1c", g1_cd)
            be1_c = cload([P, MD // P], "be1c", be1_cd)
            b2_sb = cload([P, MD2], "b2", b2_b)
            g2_sb = cload([P, MD2], "g2", g2_b)
            be2_sb = cload([P, MD2], "be2", be2_b)

            imp_all = cst.tile([P, NTT], f32, tag="imp_all")
            if lvl >= 9:
                for tt in range(NTT):
                    itt = cst.tile([P, 1], i32, tag=f"it{tt}")
                    nc.sync.dma_start(itt[:], tok[tt * P:(tt + 1) * P, :])
                    nc.gpsimd.indirect_dma_start(
                        out=imp_all[:, tt:tt + 1], out_offset=None, in_=table[:],
                        in_offset=bass.IndirectOffsetOnAxis(ap=itt[:, :1], axis=0))

            if lvl < 5:
                dout = cst.tile([P, NTT], f32, tag="dout")
                nc.any.memset(dout[:], 0.0)
                nc.sync.dma_start(out[:].rearrange("(t p) -> p t", p=P), dout[:])

            x_own = xop.tile([P, NC8, SQ], f32r, tag="x_own")

            with tc.tile_pool(name="vpool", bufs=1) as vp:
                v_sb = vp.tile([P, NKT, H], f32r, tag="v")

                with tc.tile_pool(name="xoth", bufs=1) as xot, \
                     tc.tile_pool(name="ps_mm1", bufs=6, space="PSUM") as ps1:
                    x_oth = xot.tile([P, NC8, S - SQ], f32r, tag="x_oth")

                    # ---------- phase X ----------
                    with tc.tile_pool(name="tmpx", bufs=3) as tmp:
                        for c8 in range(NC8):
                            for half, xdst in ((0, x_own), (1, x_oth)):
                                ht = tmp.tile([P, SQ], f32, tag="ht")
                                pt = tmp.tile([P, SQ], f32, tag="pt")
                                cs = half * SQ
                                nc.sync.dma_start(ht[:], hT[c8 * P:(c8 + 1) * P, cs:cs + SQ])
                                nc.sync.dma_start(pt[:], pT[c8 * P:(c8 + 1) * P, cs:cs + SQ])
                                nc.vector.tensor_tensor(out=xdst[:, c8, :], in0=ht[:],
                                                        in1=pt[:], op=OP.add)

                    # ---------- phases Q/K/V ----------
                    with tc.tile_pool(name="wqkv", bufs=2) as wst, \
                         tc.tile_pool(name="wvp", bufs=1) as wvp, \
                         tc.tile_pool(name="stgqk", bufs=4) as stg:
                        if lvl >= 1:
                            wv_sb = wvp.tile([P, NC8, H], f32r, tag="wv")
                            for db in range(H // 512):
                                nc.sync.dma_start(
                                    wv_sb[:, :, db * 512:(db + 1) * 512],
                                    wvT[:, db * 512:(db + 1) * 512]
                                    .rearrange("(c p) n -> p c n", p=P))
                        for dt in range(NC8 if lvl >= 1 else 0):
                            wq_sb = wst.tile([P, NC8, P], f32r, tag="wq")
                            nc.sync.dma_start(wq_sb[:], wqT[:, dt * P:(dt + 1) * P]
                                              .rearrange("(c p) n -> p c n", p=P))
                            psqs = [ps1.tile([P, 512], mybir.dt.float32, tag="mm512",
                                              name=f"psq{qb}") for qb in range(SQ // 512)]
                            for c8 in range(NC8):
                                for qb in range(SQ // 512):
                                    nc.tensor.matmul(psqs[qb][:],
                                                     lhsT=wq_sb[:, c8, :],
                                                     rhs=x_own[:, c8, qb * 512:(qb + 1) * 512],
                                                     start=(c8 == 0), stop=(c8 == NC8 - 1))
                            for qb in range(SQ // 512):
                                qstg = stg.tile([P, 512], f32r, tag="qstg")
                                nc.scalar.activation(qstg[:], psqs[qb][:], ACT.Identity,
                                                     bias=bq_sb[:, dt:dt + 1],
                                                     scale=INV_SQRT_HD)
                                nc.sync.dma_start(
                                    qtd[dt * P:(dt + 1) * P, qb * 512:(qb + 1) * 512],
                                    qstg[:])

                    # ---------- phase K ----------
                        for dt in range(NC8 if lvl >= 1 else 0):
                            wk_sb = wst.tile([P, NC8, P], f32r, tag="wk")
                            nc.sync.dma_start(wk_sb[:], wkT[:, dt * P:(dt + 1) * P]
                                              .rearrange("(c p) n -> p c n", p=P))
                            psks = [ps1.tile([P, 512], mybir.dt.float32, tag="mm512",
                                              name=f"psk{sb}") for sb in range(S // 512)]
                            for c8 in range(NC8):
                                for sb in range(S // 512):
                                    if sb < SQ // 512:
                                        rhs = x_own[:, c8, sb * 512:(sb + 1) * 512]
                                    else:
                                        rhs = x_oth[:, c8, (sb - SQ // 512) * 512:
                                                    (sb - SQ // 512 + 1) * 512]
                                    nc.tensor.matmul(psks[sb][:], lhsT=wk_sb[:, c8, :],
                                                     rhs=rhs,
                                                     start=(c8 == 0), stop=(c8 == NC8 - 1))
                            for sb in range(S // 512):
                                kstg = stg.tile([P, 512], f32r, tag="kstg")
                                nc.scalar.activation(kstg[:], psks[sb][:], ACT.Identity,
                                                     bias=bk_sb[:, dt:dt + 1], scale=1.0)
                                nc.sync.dma_start(
                                    ktd[dt * P:(dt + 1) * P, sb * 512:(sb + 1) * 512],
                                    kstg[:])

                    # ---------- phase V ----------
                        for tt in range(NKT if lvl >= 1 else 0):
                            psvs = [ps1.tile([P, 512], mybir.dt.float32, tag="mm512",
                                             name=f"psv{db}") for db in range(H // 512)]
                            for c8 in range(NC8):
                                if tt < NTT:
                                    lhsT = x_own[:, c8, tt * P:(tt + 1) * P]
                                else:
                                    lhsT = x_oth[:, c8, (tt - NTT) * P:(tt - NTT + 1) * P]
                                for db in range(H // 512):
                                    nc.tensor.matmul(psvs[db][:], lhsT=lhsT,
                                                     rhs=wv_sb[:, c8, db * 512:(db + 1) * 512],
                                                     start=(c8 == 0), stop=(c8 == NC8 - 1))
                            for db in range(H // 512):
                                nc.vector.tensor_tensor(
                                    out=v_sb[:, tt, db * 512:(db + 1) * 512],
                                    in0=psvs[db][:],
                                    in1=bv_sb[:, db * 512:(db + 1) * 512],
                                    op=OP.add)

                # ---------- attention (x_oth freed); ow/ctx prefetch ----------
                with tc.tile_pool(name="owp", bufs=1) as owp, \
                     tc.tile_pool(name="ctxq0", bufs=1) as cxq0:
                    if lvl >= 3:
                        ow_sb = owp.tile([P, NC8, H], f32r, tag="ow")
                        ctx_qb0 = cxq0.tile([P, NC8, 512], f32r, tag="ctx_qb0")
                    with tc.tile_pool(name="kqs", bufs=2) as kqs, \
                         tc.tile_pool(name="exps", bufs=6) as exps, \
                         tc.tile_pool(name="asml", bufs=2) as asml, \
                         tc.tile_pool(name="ps_sc", bufs=5, space="PSUM") as ps_sc, \
                         tc.tile_pool(name="ps_ctx", bufs=2, space="PSUM") as ps_ctx, \
                         tc.tile_pool(name="ps_dn", bufs=1, space="PSUM") as ps_dn:
                        for h in range(NH if lvl >= 2 else 0):
                            kt_h = kqs.tile([P, S], f32r, tag="kt_h")
                            nc.sync.dma_start(kt_h[:], ktd[h * P:(h + 1) * P, :])
                            qt_h = kqs.tile([P, SQ], f32r, tag="qt_h")
                            nc.sync.dma_start(qt_h[:], qtd[h * P:(h + 1) * P, :])

                            for qb in range(SQ // 512):
                                qsl = slice(qb * 512, (qb + 1) * 512)
                                cps = ps_ctx.tile([P, 512], mybir.dt.float32, tag="cps")
                                dn = ps_dn.tile([P, 512], mybir.dt.float32, tag="dn")
                                for kt in range(NKT):
                                    sc = ps_sc.tile([P, 512], mybir.dt.float32, tag="sc")
                                    nc.tensor.matmul(sc[:],
                                                     lhsT=kt_h[:, kt * P:(kt + 1) * P],
                                                     rhs=qt_h[:, qsl],
                                                     start=True, stop=True)
                                    ex = exps.tile([P, 512], f32r, tag="ex")
                                    nc.scalar.activation(ex[:], sc[:], ACT.Exp,
                                                         bias=kbias_sb[:, kt:kt + 1],
                                                         scale=1.0)
                                    nc.tensor.matmul(cps[:],
                                                     lhsT=v_sb[:, kt, h * P:(h + 1) * P],
                                                     rhs=ex[:],
                                                     start=(kt == 0), stop=(kt == NKT - 1))
                                    nc.tensor.matmul(dn[:],
                                                     lhsT=ones_r[:],
                                                     rhs=ex[:],
                                                     start=(kt == 0), stop=(kt == NKT - 1))
                                cpsc = asml.tile([P, 512], f32, tag="cpsc")
                                nc.vector.tensor_copy(cpsc[:], cps[:])
                                dnc = asml.tile([P, 512], f32, tag="dnc")
                                nc.vector.tensor_copy(dnc[:], dn[:])
                                rcb = asml.tile([P, 512], f32, tag="rcb")
                                with nc.allow_low_precision(reason="fp32 storage"):
                                    nc.vector.reciprocal(rcb[:], dnc[:])
                                cstg = asml.tile([P, 512], f32r, tag="cstg")
                                nc.vector.tensor_tensor(out=cstg[:], in0=cpsc[:],
                                                        in1=rcb[:], op=OP.mult)
                                nc.sync.dma_start(ctd[h * P:(h + 1) * P, qsl], cstg[:])
                                if lvl >= 3 and qb == 0:
                                    nc.sync.dma_start(
                                        ow_sb[:, :, h * P:(h + 1) * P],
                                        owT[:, h * P:(h + 1) * P]
                                        .rearrange("(c p) n -> p c n", p=P))
                                    nc.sync.dma_start(ctx_qb0[:, h, :],
                                                      ctd[h * P:(h + 1) * P, 0:512])

                    # ---------- out-projection -> atd (DRAM) ----------
                    with tc.tile_pool(name="ctxq", bufs=1) as cxq, \
                         tc.tile_pool(name="ostg", bufs=4) as ostg, \
                         tc.tile_pool(name="ps_o", bufs=6, space="PSUM") as ps_o:
                        if lvl >= 3:
                            ctx_qb1 = cxq.tile([P, NC8, 512], f32r, tag="ctx_qb1")
                            for c8 in range(NC8):
                                nc.sync.dma_start(ctx_qb1[:, c8, :],
                                                  ctd[c8 * P:(c8 + 1) * P, 512:1024])
                            ctx_qbs = [ctx_qb0, ctx_qb1]
                            for qb in range(SQ // 512):
                                for dt in range(NC8):
                                    pso = ps_o.tile([P, 512], mybir.dt.float32,
                                                    tag="mm512", name="pso")
                                    for c8 in range(NC8):
                                        nc.tensor.matmul(
                                            pso[:],
                                            lhsT=ow_sb[:, c8, dt * P:(dt + 1) * P],
                                            rhs=ctx_qbs[qb][:, c8, :],
                                            start=(c8 == 0), stop=(c8 == NC8 - 1))
                                    osg = ostg.tile([P, 512], f32r, tag="osg")
                                    nc.scalar.activation(osg[:], pso[:],
                                                         ACT.Identity,
                                                         bias=ob_sb[:, dt:dt + 1],
                                                         scale=1.0)
                                    nc.sync.dma_start(
                                        atd[dt * P:(dt + 1) * P,
                                            qb * 512:(qb + 1) * 512], osg[:])

            # ---------- meta MLP ----------
            with tc.tile_pool(name="mw", bufs=1) as mw, \
                 tc.tile_pool(name="msml", bufs=3) as sml, \
                 tc.tile_pool(name="attq", bufs=1) as atq, \
                 tc.tile_pool(name="ps_m", bufs=6, space="PSUM") as ps2:
                if lvl >= 4:
                    w1_sb = mw.tile([P, 2 * NC8, MD], f32r, tag="w1")
                    nc.sync.dma_start(w1_sb[:],
                                      w1T[:].rearrange("(c p) n -> p c n", p=P))
                    w2_sb = mw.tile([P, MD // P, MD2], f32r, tag="w2")
                    nc.sync.dma_start(w2_sb[:],
                                      w2T[:].rearrange("(c p) n -> p c n", p=P))
                    res_sb = mw.tile([P, NTT], f32, tag="res")

                    # ---- h1 in feature-major: h1preT [256, SQ] ----
                    att_qbs = []
                    for qb in range(SQ // 512):
                        att_qb = atq.tile([P, NC8, 512], f32r, tag=f"att_qb{qb}")
                        for c8 in range(NC8):
                            nc.sync.dma_start(
                                att_qb[:, c8, :],
                                atd[c8 * P:(c8 + 1) * P, qb * 512:(qb + 1) * 512])
                        att_qbs.append(att_qb)
                    NFT = MD // P      # 2 feature tiles of h1
                    h1p = mw.tile([P, NFT, SQ], f32r, tag="h1p")
                    h1sq = mw.tile([P, NFT, SQ], f32r, tag="h1x")
                    h1n = mw.tile([P, NFT, SQ], f32r, tag="h1x", name="h1n")
                    stat = mw.tile([P, 3, SQ], f32, tag="stat")
                    nmean, work, m2r = stat[:, 0, :], stat[:, 1, :], stat[:, 2, :]
                    ex2m = varm = rstd = work

                    for ft in range(NFT):
                        for qb in range(SQ // 512):
                            psf_t = ps2.tile([P, 512], mybir.dt.float32,
                                             tag="mm512", name="psf")
                            for c16 in range(2 * NC8):
                                if c16 < NC8:
                                    rhs = x_own[:, c16, qb * 512:(qb + 1) * 512]
                                else:
                                    rhs = att_qbs[qb][:, c16 - NC8, :]
                                nc.tensor.matmul(
                                    psf_t[:],
                                    lhsT=w1_sb[:, c16, ft * P:(ft + 1) * P],
                                    rhs=rhs,
                                    start=(c16 == 0), stop=(c16 == 2 * NC8 - 1))
                            nc.scalar.activation(
                                h1p[:, ft, qb * 512:(qb + 1) * 512], psf_t[:],
                                ACT.Identity, bias=b1_c[:, ft:ft + 1], scale=1.0)
                    for qb in range(SQ // 512):
                        qsl = slice(qb * 512, (qb + 1) * 512)
                        for ft in range(NFT):
                            nc.vector.tensor_tensor(out=h1sq[:, ft, qsl],
                                                    in0=h1p[:, ft, qsl],
                                                    in1=h1p[:, ft, qsl], op=OP.mult)
                        psA = ps2.tile([P, 512], mybir.dt.float32, tag="mm512",
                                       name="psA")
                        psB = ps2.tile([P, 512], mybir.dt.float32, tag="mm512",
                                       name="psB")
                        for ft in range(NFT):
                            nc.tensor.matmul(psA[:], lhsT=ones_r[:],
                                             rhs=h1p[:, ft, qsl],
                                             start=(ft == 0), stop=(ft == NFT - 1))
                        for ft in range(NFT):
                            nc.tensor.matmul(psB[:], lhsT=ones_r[:],
                                             rhs=h1sq[:, ft, qsl],
                                             start=(ft == 0), stop=(ft == NFT - 1))
                        nc.vector.tensor_scalar_mul(nmean[:, qsl], psA[:],
                                                    -1.0 / MD)
                        nc.vector.tensor_scalar_mul(ex2m[:, qsl], psB[:], 1.0 / MD)
                        nc.vector.tensor_tensor(out=m2r[:, qsl], in0=nmean[:, qsl],
                                                in1=nmean[:, qsl], op=OP.mult)
                        nc.vector.tensor_tensor(out=work[:, qsl], in0=work[:, qsl],
                                                in1=m2r[:, qsl], op=OP.subtract)
                        # rstd = exp(-0.5 * ln(var + eps)) on ACT (fast path)
                        nc.scalar.activation(varm[:, qsl], varm[:, qsl], ACT.Ln,
                                             bias=eps_sb[:, 0:1], scale=1.0)
                        nc.scalar.activation(rstd[:, qsl], varm[:, qsl], ACT.Exp,
                                             bias=0.0, scale=-0.5)
                        for ft in range(NFT):
                            nc.vector.tensor_tensor(out=h1n[:, ft, qsl],
                                                    in0=h1p[:, ft, qsl],
                                                    in1=nmean[:, qsl], op=OP.add)
                            nc.vector.tensor_tensor(out=h1n[:, ft, qsl],
                                                    in0=h1n[:, ft, qsl],
                                                    in1=rstd[:, qsl], op=OP.mult)
                            nc.scalar.activation(h1n[:, ft, qsl], h1n[:, ft, qsl],
                                                 ACT.Relu, bias=be1_c[:, ft:ft + 1],
                                                 scale=g1_c[:, ft:ft + 1])

                # ---- h2 + batched LN2/final across all tiles ----
                hb2_all = mw.tile([P, NTT, MD2], f32, tag="hb2_all")
                for tt in range(NTT if lvl >= 5 else 0):
                    ph2_t = ps2.tile([P, 512], mybir.dt.float32, tag="mm512",
                                     name="ph2")
                    ph2 = ph2_t[:, :MD2]
                    for ft in range(MD // P):
                        nc.tensor.matmul(ph2, lhsT=h1n[:, ft, tt * P:(tt + 1) * P],
                                         rhs=w2_sb[:, ft, :],
                                         start=(ft == 0), stop=(ft == MD // P - 1))
                    nc.vector.scalar_tensor_tensor(out=hb2_all[:, tt, :], in0=ph2,
                                                   scalar=1.0, in1=b2_sb[:],
                                                   op0=OP.mult, op1=OP.add)
                if lvl >= 5:
                    F2 = float(MD2)
                    sums2 = sml.tile([P, NTT], f32, tag="sums2")
                    nc.vector.reduce_sum(sums2[:], hb2_all[:],
                                         axis=mybir.AxisListType.X)
                    msq = sml.tile([P, NTT, MD2], f32, tag="msq")
                    ssq2 = sml.tile([P, NTT], f32, tag="ssq2")
                    nc.vector.tensor_tensor(out=msq[:], in0=hb2_all[:],
                                            in1=hb2_all[:], op=OP.mult)
                    nc.vector.reduce_sum(ssq2[:], msq[:], axis=mybir.AxisListType.X)
                    nm2 = sml.tile([P, NTT], f32, tag="nm2")
                    nc.vector.tensor_scalar_mul(nm2[:], sums2[:], -1.0 / F2)
                    ex22 = sml.tile([P, NTT], f32, tag="ex22")
                    nc.vector.tensor_scalar_mul(ex22[:], ssq2[:], 1.0 / F2)
                    mm2 = sml.tile([P, NTT], f32, tag="mm2")
                    nc.vector.tensor_tensor(out=mm2[:], in0=nm2[:], in1=nm2[:],
                                            op=OP.mult)
                    var2 = sml.tile([P, NTT], f32, tag="var2")
                    nc.vector.tensor_tensor(out=var2[:], in0=ex22[:], in1=mm2[:],
                                            op=OP.subtract)
                    std2 = sml.tile([P, NTT], f32, tag="std2")
                    nc.scalar.activation(std2[:], var2[:], ACT.Sqrt,
                                         bias=eps_sb[:, 0:1], scale=1.0)
                    rstd2 = sml.tile([P, NTT], f32, tag="rstd2")
                    nc.vector.reciprocal(rstd2[:], std2[:])
                    t1a = sml.tile([P, NTT, MD2], f32, tag="t1a")
                    nc.vector.tensor_tensor(
                        out=t1a[:], in0=hb2_all[:],
                        in1=nm2[:, :, None].to_broadcast([P, NTT, MD2]),
                        op=OP.add)
                    nc.vector.tensor_tensor(
                        out=t1a[:], in0=t1a[:],
                        in1=rstd2[:, :, None].to_broadcast([P, NTT, MD2]),
                        op=OP.mult)
                    nc.vector.tensor_tensor(
                        out=t1a[:], in0=t1a[:],
                        in1=g2_sb[:, None, :].to_broadcast([P, NTT, MD2]),
                        op=OP.mult)
                    nc.vector.tensor_tensor(
                        out=t1a[:], in0=t1a[:],
                        in1=be2_sb[:, None, :].to_broadcast([P, NTT, MD2]),
                        op=OP.add)
                    nc.vector.tensor_scalar_max(t1a[:], t1a[:], 0.0)
                    nc.vector.tensor_tensor(
                        out=t1a[:], in0=t1a[:],
                        in1=w3_sb[:, None, :].to_broadcast([P, NTT, MD2]),
                        op=OP.mult)
                    base8 = sml.tile([P, NTT], f32, tag="base8")
                    nc.vector.reduce_sum(base8[:], t1a[:], axis=mybir.AxisListType.X)
                    nc.vector.tensor_tensor(
                        out=base8[:], in0=base8[:],
                        in1=b3_sb[:, 0:1].to_broadcast([P, NTT]), op=OP.add)
                    imp1a = sml.tile([P, NTT], f32, tag="imp1a")
                    nc.vector.tensor_scalar_add(imp1a[:], imp_all[:], 1.0)
                    nc.vector.tensor_tensor(out=base8[:], in0=base8[:],
                                            in1=imp1a[:], op=OP.mult)
                    nc.vector.tensor_scalar(base8[:], base8[:], MAX_W, MIN_W,
                                            op0=OP.min, op1=OP.max)
                    nc.vector.tensor_tensor(out=res_sb[:], in0=base8[:],
                                            in1=maskf_sb[:], op=OP.mult)
                    nc.sync.dma_start(out[:].rearrange("(t p) -> p t", p=P),
                                      res_sb[:])

    nc.compile()
    return nc


def _get_program():
    import os
    stop = os.environ.get("KB_STOP") or None
    key = ("nc", stop)
    if key not in _CACHE:
        _CACHE[key] = _build(stop)
    return _CACHE[key]


def _prep_in_maps(inputs):
    hidden = np.ascontiguousarray(np.asarray(inputs["hidden_states"], dtype=np.float32))
    token_ids = np.asarray(inputs["token_ids"], dtype=np.int32)
    mask = np.asarray(inputs["attention_mask"]).astype(bool)
    pos = np.asarray(inputs["pos_embed"], dtype=np.float32)
    in_proj_w = np.asarray(inputs["in_proj_w"], dtype=np.float32)
    in_proj_b = np.asarray(inputs["in_proj_b"], dtype=np.float32)
    out_w = np.asarray(inputs["out_w"], dtype=np.float32)
    out_b = np.asarray(inputs["out_b"], dtype=np.float32)
    w1 = np.asarray(inputs["w1"], dtype=np.float32)
    b1 = np.asarray(inputs["b1"], dtype=np.float32)
    g1 = np.asarray(inputs["g1"], dtype=np.float32)
    beta1 = np.asarray(inputs["beta1"], dtype=np.float32)
    w2 = np.asarray(inputs["w2"], dtype=np.float32)
    b2 = np.asarray(inputs["b2"], dtype=np.float32)
    g2 = np.asarray(inputs["g2"], dtype=np.float32)
    beta2 = np.asarray(inputs["beta2"], dtype=np.float32)
    w3 = np.asarray(inputs["w3"], dtype=np.float32)
    b3 = np.asarray(inputs["b3"], dtype=np.float32)
    table = np.asarray(inputs["importance_table"], dtype=np.float32)

    B, S_, H_ = hidden.shape
    assert (B, S_, H_) == (4, S, H), (B, S_, H_)

    posT = np.ascontiguousarray(pos[0].T)                      # [H, S]
    wqT = np.ascontiguousarray(in_proj_w[0:H].T)               # [H, H]
    wkT = np.ascontiguousarray(in_proj_w[H:2 * H].T)
    wvT = np.ascontiguousarray(in_proj_w[2 * H:3 * H].T)
    bq = in_proj_b[0:H]
    bk = in_proj_b[H:2 * H]
    bv = in_proj_b[2 * H:3 * H]
    owT = np.ascontiguousarray(out_w.T)
    w1T = np.ascontiguousarray(w1.T)                           # [2H, MD]
    w2T = np.ascontiguousarray(w2.T)                           # [MD, MD2]

    def cmaj(v):   # [H] -> [128, H/128] partition-major (column dt holds v[dt*128+p])
        return np.ascontiguousarray(v.reshape(-1, P).T)

    def bcast(v):  # [F] -> [128, F]
        return np.ascontiguousarray(np.broadcast_to(v[None, :], (P, v.shape[0])))

    shared = {
        "wqT": wqT, "wkT": wkT, "wvT": wvT,
        "bq_c": cmaj(bq), "bk_c": cmaj(bk), "bv_b": bcast(bv),
        "owT": owT, "ob_c": cmaj(out_b),
        "w1T": w1T, "b1_cd": cmaj(b1), "g1_cd": cmaj(g1), "be1_cd": cmaj(beta1),
        "w2T": w2T, "b2_b": bcast(b2), "g2_b": bcast(g2), "be2_b": bcast(beta2),
        "w3_b": bcast(w3[0]), "b3_c": np.full((P, 1), b3[0], dtype=np.float32),
        "table": np.ascontiguousarray(table[:, None]),
    }

    in_maps = []
    for c in range(8):
        b = c // 2
        half = c % 2
        own = slice(half * SQ, (half + 1) * SQ)
        oth = slice((1 - half) * SQ, (2 - half) * SQ)
        hT_b = hidden[b].T                                     # [H, S] view
        # arrange so own half occupies columns [0, SQ)
        hT_arr = np.ascontiguousarray(
            np.concatenate([hT_b[:, own], hT_b[:, oth]], axis=1))
        pT_arr = np.ascontiguousarray(
            np.concatenate([posT[:, own], posT[:, oth]], axis=1))
        kb = np.where(mask[b], 0.0, -1e9).astype(np.float32)
        kb_arr = np.concatenate([kb[own], kb[oth]])            # match column remap
        m = {
            "hT": hT_arr, "pT": pT_arr,
            "kbias": np.ascontiguousarray(kb_arr.reshape(-1, P).T),
            "maskf": np.ascontiguousarray(
                mask[b, own].astype(np.float32).reshape(-1, P).T),
            "tok": np.ascontiguousarray(token_ids[b, own][:, None]),
        }
        m.update(shared)
        in_maps.append(m)
    return in_maps


def _assemble(res):
    full = np.zeros((4, S), dtype=np.float32)
    for c in range(8):
        b = c // 2
        half = c % 2
        full[b, half * SQ:(half + 1) * SQ] = res.results[c]["out"]
    return full


def kernel(**inputs) -> np.ndarray:
    from concourse.bass_utils import run_bass_kernel_spmd
    in_maps = _prep_in_maps(inputs)
    nc = _get_program()
    res = run_bass_kernel_spmd(nc, in_maps, list(range(8)))
    return _assemble(res)


def run_traced(inputs, **kwargs):
    from concourse.bass_utils import run_bass_kernel_spmd
    in_maps = _prep_in_maps(inputs)
    nc = _get_program()
    return run_bass_kernel_spmd(nc, in_maps, list(range(8)), trace=True, **kwargs)



# revision 3
# speedup vs baseline: 1.2107x; 1.2107x over previous
"""Trainium2 Bass kernel for EnhancedMetaWeightNetwork.

Full (unsharded) inputs in, full output out. Internally: 8 NeuronCores,
core c handles batch b = c // 2 and query-row half c % 2 (1024 rows).
Attention K/V are computed per-core for the full sequence of the core's
batch (duplicated across the 2 cores sharing a batch; no collectives).

v2 layout strategy (bf16 storage, fp32 PSUM accumulation):
  - x = hidden + pos_embed precomputed on host; uploaded as bf16 x^T.
  - out_w folded into w1 on host (w1eff = w1[:, H:] @ out_w), so the
    attention out-projection disappears; h1 = w1x @ x + w1eff @ ctx.
  - everything SBUF-resident (bf16 halves footprints): no DRAM scratch.
  - scoresT [key, query] per head; softmax denominator via a bf16
    pairwise-add tree on the vector engine + one ones-matmul (instead of
    16 PE ones-matmuls per (head, qblock)).
  - importance lookup via indirect DMA gather from the vocab table.
"""

import numpy as np

H = 1024
NH = 8
HD = 128           # head dim
S = 2048           # keys / full sequence
SQ = 1024          # own query rows per core
MD = 256           # meta dim
MD2 = 128
VOCAB = 32000
MIN_W, MAX_W = 0.1, 5.0
LN_EPS = 1e-5
P = 128
INV_SQRT_HD = 1.0 / np.sqrt(np.float32(HD))

_CACHE = {}


def _build():
    import concourse.bass as bass
    import concourse.mybir as mybir
    import concourse.tile as tile
    from concourse import bacc

    f32 = mybir.dt.float32
    bf16 = mybir.dt.bfloat16
    i32 = mybir.dt.int32
    OP = mybir.AluOpType
    ACT = mybir.ActivationFunctionType

    nc = bacc.Bacc("TRN2", target_bir_lowering=False, debug=False,
                   enable_asserts=False, num_devices=8)

    # ---------------- DRAM parameters ----------------
    dp = nc.declare_dram_parameter
    xT = dp("xT", [H, S], bf16, isOutput=False)           # (hidden+pos)[b].T, own half first
    wqT = dp("wqT", [H, H], bf16, isOutput=False)         # in_proj_w[0:H].T
    wkT = dp("wkT", [H, H], bf16, isOutput=False)
    wvT = dp("wvT", [H, H], bf16, isOutput=False)
    w1cT = dp("w1cT", [2 * H, MD], bf16, isOutput=False)  # [w1x | w1a@out_w].T
    w2T = dp("w2T", [MD, MD2], bf16, isOutput=False)
    bq_c = dp("bq_c", [P, H // P], f32, isOutput=False)   # bias/sqrt(hd), partition-major
    bk_c = dp("bk_c", [P, H // P], f32, isOutput=False)
    bv_b = dp("bv_b", [P, H], f32, isOutput=False)        # bias bcast over partitions
    b1_cd = dp("b1_cd", [P, MD // P], f32, isOutput=False)
    g1_cd = dp("g1_cd", [P, MD // P], f32, isOutput=False)
    be1_cd = dp("be1_cd", [P, MD // P], f32, isOutput=False)
    b2_b = dp("b2_b", [P, MD2], f32, isOutput=False)
    g2_b = dp("g2_b", [P, MD2], f32, isOutput=False)
    be2_b = dp("be2_b", [P, MD2], f32, isOutput=False)
    w3_b = dp("w3_b", [P, MD2], f32, isOutput=False)
    b3_c = dp("b3_c", [P, 1], f32, isOutput=False)
    kbias = dp("kbias", [P, S // P], f32, isOutput=False)  # attn mask bias per key
    maskf = dp("maskf", [P, SQ // P], f32, isOutput=False)
    tok = dp("tok", [SQ, 1], i32, isOutput=False)
    table = dp("table", [VOCAB, 1], f32, isOutput=False)
    out = dp("out", [SQ], f32, isOutput=True)

    NKT = S // P          # 16 key tiles
    NC8 = H // P          # 8 feature chunks
    NTT = SQ // P         # 8 own token tiles
    NFT = MD // P         # 2 feature tiles of h1

    with tile.TileContext(nc) as tc:
        with tc.tile_pool(name="const", bufs=1) as cst, \
             tc.tile_pool(name="persist", bufs=1) as pa:

            # ---------------- constants ----------------
            ones_f = cst.tile([P, P], f32, tag="ones_f")
            nc.any.memset(ones_f[:], 1.0)
            ones_b = cst.tile([P, P], bf16, tag="ones_b")
            nc.vector.tensor_copy(ones_b[:], ones_f[:])
            eps_sb = cst.tile([P, 1], f32, tag="eps")
            nc.any.memset(eps_sb[:], LN_EPS)

            def cload(shape, tag, src):
                t = cst.tile(shape, f32, tag=tag)
                nc.sync.dma_start(t[:], src[:])
                return t

            kbias_sb = cload([P, NKT], "kbias", kbias)
            maskf_sb = cload([P, NTT], "maskf", maskf)
            b3_sb = cload([P, 1], "b3", b3_c)
            w3_sb = cload([P, MD2], "w3", w3_b)
            bq_sb = cload([P, NC8], "bq", bq_c)
            bk_sb = cload([P, NC8], "bk", bk_c)
            bv_sb = cload([P, H], "bv", bv_b)
            b1_c = cload([P, NFT], "b1c", b1_cd)
            g1_c = cload([P, NFT], "g1c", g1_cd)
            be1_c = cload([P, NFT], "be1c", be1_cd)
            b2_sb = cload([P, MD2], "b2", b2_b)
            g2_sb = cload([P, MD2], "g2", g2_b)
            be2_sb = cload([P, MD2], "be2", be2_b)

            # importance gather (gpsimd; independent of everything else)
            imp_all = cst.tile([P, NTT], f32, tag="imp_all")
            for tt in range(NTT):
                itt = cst.tile([P, 1], i32, tag=f"it{tt}")
                nc.sync.dma_start(itt[:], tok[tt * P:(tt + 1) * P, :])
                nc.gpsimd.indirect_dma_start(
                    out=imp_all[:, tt:tt + 1], out_offset=None, in_=table[:],
                    in_offset=bass.IndirectOffsetOnAxis(ap=itt[:, :1], axis=0))

            # ---- long-lived activations ----
            x_sb = pa.tile([P, NC8, S], bf16, tag="x")       # full x^T (own cols first)
            ctx_sb = pa.tile([P, NC8, SQ], bf16, tag="ctx")  # ctx^T (head-major)
            w1c_sb = pa.tile([P, 2 * NC8, MD], bf16, tag="w1c")
            h1p = pa.tile([P, NFT, SQ], bf16, tag="h1p")
            h1n = pa.tile([P, NFT, SQ], bf16, tag="h1n")
            res_sb = pa.tile([P, NTT], f32, tag="res")

            for c8 in range(NC8):
                nc.sync.dma_start(x_sb[:, c8, :], xT[c8 * P:(c8 + 1) * P, :])

            with tc.tile_pool(name="qkv", bufs=1) as qkv:
                q_sb = qkv.tile([P, NH, SQ], bf16, tag="q")
                k_sb = qkv.tile([P, NH, S], bf16, tag="k")
                v_sb = qkv.tile([P, NKT, H], bf16, tag="v")

                # ---------- Q / K / V projections ----------
                with tc.tile_pool(name="wst", bufs=2) as wst, \
                     tc.tile_pool(name="wvp", bufs=1) as wvp, \
                     tc.tile_pool(name="ps_mm1", bufs=6, space="PSUM") as ps1:
                    wv_sb = wvp.tile([P, NC8, H], bf16, tag="wv")
                    for db in range(H // 512):
                        nc.sync.dma_start(
                            wv_sb[:, :, db * 512:(db + 1) * 512],
                            wvT[:, db * 512:(db + 1) * 512]
                            .rearrange("(c p) n -> p c n", p=P))
                    nc.sync.dma_start(w1c_sb[:],
                                      w1cT[:].rearrange("(c p) n -> p c n", p=P))

                    for dt in range(NC8):
                        wq_sb = wst.tile([P, NC8, P], bf16, tag="wq")
                        nc.sync.dma_start(wq_sb[:], wqT[:, dt * P:(dt + 1) * P]
                                          .rearrange("(c p) n -> p c n", p=P))
                        psqs = [ps1.tile([P, 512], mybir.dt.float32, tag="mm512",
                                          name=f"psq{qb}") for qb in range(SQ // 512)]
                        for c8 in range(NC8):
                            for qb in range(SQ // 512):
                                nc.tensor.matmul(psqs[qb][:],
                                                 lhsT=wq_sb[:, c8, :],
                                                 rhs=x_sb[:, c8, qb * 512:(qb + 1) * 512],
                                                 start=(c8 == 0), stop=(c8 == NC8 - 1))
                        for qb in range(SQ // 512):
                            nc.scalar.activation(q_sb[:, dt, qb * 512:(qb + 1) * 512],
                                                 psqs[qb][:], ACT.Identity,
                                                 bias=bq_sb[:, dt:dt + 1],
                                                 scale=INV_SQRT_HD)

                    for dt in range(NC8):
                        wk_sb = wst.tile([P, NC8, P], bf16, tag="wk")
                        nc.sync.dma_start(wk_sb[:], wkT[:, dt * P:(dt + 1) * P]
                                          .rearrange("(c p) n -> p c n", p=P))
                        psks = [ps1.tile([P, 512], mybir.dt.float32, tag="mm512",
                                          name=f"psk{sb}") for sb in range(S // 512)]
                        for c8 in range(NC8):
                            for sb in range(S // 512):
                                nc.tensor.matmul(psks[sb][:], lhsT=wk_sb[:, c8, :],
                                                 rhs=x_sb[:, c8, sb * 512:(sb + 1) * 512],
                                                 start=(c8 == 0), stop=(c8 == NC8 - 1))
                        for sb in range(S // 512):
                            nc.scalar.activation(k_sb[:, dt, sb * 512:(sb + 1) * 512],
                                                 psks[sb][:], ACT.Identity,
                                                 bias=bk_sb[:, dt:dt + 1], scale=1.0)

                    for tt in range(NKT):
                        psvs = [ps1.tile([P, 512], mybir.dt.float32, tag="mm512",
                                         name=f"psv{db}") for db in range(H // 512)]
                        for c8 in range(NC8):
                            lhsT = x_sb[:, c8, tt * P:(tt + 1) * P]
                            for db in range(H // 512):
                                nc.tensor.matmul(psvs[db][:], lhsT=lhsT,
                                                 rhs=wv_sb[:, c8, db * 512:(db + 1) * 512],
                                                 start=(c8 == 0), stop=(c8 == NC8 - 1))
                        for db in range(H // 512):
                            nc.vector.tensor_tensor(
                                out=v_sb[:, tt, db * 512:(db + 1) * 512],
                                in0=psvs[db][:],
                                in1=bv_sb[:, db * 512:(db + 1) * 512],
                                op=OP.add)

                # ---------- attention ----------
                with tc.tile_pool(name="exps", bufs=6) as exps, \
                     tc.tile_pool(name="dtree", bufs=2) as trp, \
                     tc.tile_pool(name="rcps", bufs=2) as rcps, \
                     tc.tile_pool(name="ps_sc", bufs=4, space="PSUM") as ps_sc, \
                     tc.tile_pool(name="ps_ctx", bufs=2, space="PSUM") as ps_ctx, \
                     tc.tile_pool(name="ps_dn", bufs=2, space="PSUM") as ps_dn:
                    for h in range(NH):
                        for qb in range(SQ // 512):
                            qsl = slice(qb * 512, (qb + 1) * 512)
                            cps = ps_ctx.tile([P, 512], mybir.dt.float32, tag="cps")
                            # softmax-denominator pairwise tree accumulators
                            accs = [trp.tile([P, 512], bf16, tag=f"acc{a}",
                                             name=f"acc{a}") for a in range(4)]
                            exs = []
                            for kt in range(NKT):
                                sc = ps_sc.tile([P, 512], mybir.dt.float32, tag="sc")
                                nc.tensor.matmul(sc[:],
                                                 lhsT=k_sb[:, h, kt * P:(kt + 1) * P],
                                                 rhs=q_sb[:, h, qsl],
                                                 start=True, stop=True)
                                ex = exps.tile([P, 512], bf16, tag="ex")
                                nc.scalar.activation(ex[:], sc[:], ACT.Exp,
                                                     bias=kbias_sb[:, kt:kt + 1],
                                                     scale=1.0)
                                nc.tensor.matmul(cps[:],
                                                 lhsT=v_sb[:, kt, h * P:(h + 1) * P],
                                                 rhs=ex[:],
                                                 start=(kt == 0), stop=(kt == NKT - 1))
                                exs.append(ex)
                                if kt % 4 == 3:
                                    g = kt // 4
                                    t0 = trp.tile([P, 512], bf16, tag="t0")
                                    t1 = trp.tile([P, 512], bf16, tag="t1")
                                    nc.vector.tensor_tensor(out=t0[:], in0=exs[0][:],
                                                            in1=exs[1][:], op=OP.add)
                                    nc.vector.tensor_tensor(out=t1[:], in0=exs[2][:],
                                                            in1=exs[3][:], op=OP.add)
                                    nc.vector.tensor_tensor(out=accs[g][:], in0=t0[:],
                                                            in1=t1[:], op=OP.add)
                                    exs = []
                            m0 = trp.tile([P, 512], bf16, tag="m0")
                            m1 = trp.tile([P, 512], bf16, tag="m1")
                            dnt = trp.tile([P, 512], bf16, tag="dnt")
                            nc.vector.tensor_tensor(out=m0[:], in0=accs[0][:],
                                                    in1=accs[1][:], op=OP.add)
                            nc.vector.tensor_tensor(out=m1[:], in0=accs[2][:],
                                                    in1=accs[3][:], op=OP.add)
                            nc.vector.tensor_tensor(out=dnt[:], in0=m0[:],
                                                    in1=m1[:], op=OP.add)
                            dnf = ps_dn.tile([P, 512], mybir.dt.float32, tag="dnf")
                            nc.tensor.matmul(dnf[:], lhsT=ones_b[:], rhs=dnt[:],
                                             start=True, stop=True)
                            rcb = rcps.tile([P, 512], bf16, tag="rcb")
                            with nc.allow_low_precision(reason="bf16 softmax scale"):
                                nc.vector.reciprocal(rcb[:], dnf[:])
                            nc.vector.tensor_tensor(out=ctx_sb[:, h, qsl],
                                                    in0=cps[:], in1=rcb[:],
                                                    op=OP.mult)

            # ---------- meta MLP ----------
            with tc.tile_pool(name="mw", bufs=1) as mw, \
                 tc.tile_pool(name="msml", bufs=2) as sml, \
                 tc.tile_pool(name="ps_m", bufs=6, space="PSUM") as ps2:
                w2_sb = mw.tile([P, NFT, MD2], bf16, tag="w2")
                nc.sync.dma_start(w2_sb[:],
                                  w2T[:].rearrange("(c p) n -> p c n", p=P))

                # ---- h1 in feature-major: h1preT [256, SQ] ----
                for ft in range(NFT):
                    for qb in range(SQ // 512):
                        psf_t = ps2.tile([P, 512], mybir.dt.float32,
                                         tag="mm512", name="psf")
                        for j in range(2 * NC8):
                            if j < NC8:
                                rhs = x_sb[:, j, qb * 512:(qb + 1) * 512]
                            else:
                                rhs = ctx_sb[:, j - NC8, qb * 512:(qb + 1) * 512]
                            nc.tensor.matmul(
                                psf_t[:],
                                lhsT=w1c_sb[:, j, ft * P:(ft + 1) * P],
                                rhs=rhs,
                                start=(j == 0), stop=(j == 2 * NC8 - 1))
                        nc.scalar.activation(
                            h1p[:, ft, qb * 512:(qb + 1) * 512], psf_t[:],
                            ACT.Identity, bias=b1_c[:, ft:ft + 1], scale=1.0)
                for qb in range(SQ // 512):
                    qsl = slice(qb * 512, (qb + 1) * 512)
                    h1sqs = []
                    for ft in range(NFT):
                        h1sq = sml.tile([P, 512], bf16, tag=f"h1sq{ft}")
                        nc.vector.tensor_tensor(out=h1sq[:], in0=h1p[:, ft, qsl],
                                                in1=h1p[:, ft, qsl], op=OP.mult)
                        h1sqs.append(h1sq)
                    psA = ps2.tile([P, 512], mybir.dt.float32, tag="mm512",
                                   name="psA")
                    psB = ps2.tile([P, 512], mybir.dt.float32, tag="mm512",
                                   name="psB")
                    for ft in range(NFT):
                        nc.tensor.matmul(psA[:], lhsT=ones_b[:],
                                         rhs=h1p[:, ft, qsl],
                                         start=(ft == 0), stop=(ft == NFT - 1))
                    for ft in range(NFT):
                        nc.tensor.matmul(psB[:], lhsT=ones_b[:],
                                         rhs=h1sqs[ft][:],
                                         start=(ft == 0), stop=(ft == NFT - 1))
                    nmean = sml.tile([P, 512], f32, tag="nmean")
                    ex2m = sml.tile([P, 512], f32, tag="ex2m")
                    m2r = sml.tile([P, 512], f32, tag="m2r")
                    nc.vector.tensor_scalar_mul(nmean[:], psA[:], -1.0 / MD)
                    nc.vector.tensor_scalar_mul(ex2m[:], psB[:], 1.0 / MD)
                    nc.vector.tensor_tensor(out=m2r[:], in0=nmean[:],
                                            in1=nmean[:], op=OP.mult)
                    nc.vector.tensor_tensor(out=ex2m[:], in0=ex2m[:],
                                            in1=m2r[:], op=OP.subtract)
                    # rstd = exp(-0.5 * ln(var + eps)) on ACT (fast path)
                    nc.scalar.activation(ex2m[:], ex2m[:], ACT.Ln,
                                         bias=eps_sb[:, 0:1], scale=1.0)
                    rstd = sml.tile([P, 512], bf16, tag="rstd")
                    with nc.allow_low_precision(reason="bf16 layernorm scale"):
                        nc.scalar.activation(rstd[:], ex2m[:], ACT.Exp,
                                             bias=0.0, scale=-0.5)
                    for ft in range(NFT):
                        h1c = sml.tile([P, 512], bf16, tag=f"h1c{ft}")
                        nc.vector.tensor_tensor(out=h1c[:], in0=h1p[:, ft, qsl],
                                                in1=nmean[:], op=OP.add)
                        nc.vector.tensor_tensor(out=h1c[:], in0=h1c[:],
                                                in1=rstd[:], op=OP.mult)
                        nc.scalar.activation(h1n[:, ft, qsl], h1c[:],
                                             ACT.Relu, bias=be1_c[:, ft:ft + 1],
                                             scale=g1_c[:, ft:ft + 1])

                # ---- h2 + batched LN2/final across all tiles ----
                hb2_all = mw.tile([P, NTT, MD2], f32, tag="hb2_all")
                for tt in range(NTT):
                    ph2_t = ps2.tile([P, 512], mybir.dt.float32, tag="mm512",
                                     name="ph2")
                    ph2 = ph2_t[:, :MD2]
                    for ft in range(NFT):
                        nc.tensor.matmul(ph2, lhsT=h1n[:, ft, tt * P:(tt + 1) * P],
                                         rhs=w2_sb[:, ft, :],
                                         start=(ft == 0), stop=(ft == NFT - 1))
                    nc.vector.scalar_tensor_tensor(out=hb2_all[:, tt, :], in0=ph2,
                                                   scalar=1.0, in1=b2_sb[:],
                                                   op0=OP.mult, op1=OP.add)
                F2 = float(MD2)
                sums2 = sml.tile([P, NTT], f32, tag="sums2")
                nc.vector.reduce_sum(sums2[:], hb2_all[:],
                                     axis=mybir.AxisListType.X)
                msq = sml.tile([P, NTT, MD2], f32, tag="msq")
                ssq2 = sml.tile([P, NTT], f32, tag="ssq2")
                nc.vector.tensor_tensor(out=msq[:], in0=hb2_all[:],
                                        in1=hb2_all[:], op=OP.mult)
                nc.vector.reduce_sum(ssq2[:], msq[:], axis=mybir.AxisListType.X)
                nm2 = sml.tile([P, NTT], f32, tag="nm2")
                nc.vector.tensor_scalar_mul(nm2[:], sums2[:], -1.0 / F2)
                ex22 = sml.tile([P, NTT], f32, tag="ex22")
                nc.vector.tensor_scalar_mul(ex22[:], ssq2[:], 1.0 / F2)
                mm2 = sml.tile([P, NTT], f32, tag="mm2")
                nc.vector.tensor_tensor(out=mm2[:], in0=nm2[:], in1=nm2[:],
                                        op=OP.mult)
                var2 = sml.tile([P, NTT], f32, tag="var2")
                nc.vector.tensor_tensor(out=var2[:], in0=ex22[:], in1=mm2[:],
                                        op=OP.subtract)
                std2 = sml.tile([P, NTT], f32, tag="std2")
                nc.scalar.activation(std2[:], var2[:], ACT.Sqrt,
                                     bias=eps_sb[:, 0:1], scale=1.0)
                rstd2 = sml.tile([P, NTT], f32, tag="rstd2")
                nc.vector.reciprocal(rstd2[:], std2[:])
                t1a = sml.tile([P, NTT, MD2], f32, tag="t1a")
                nc.vector.tensor_tensor(
                    out=t1a[:], in0=hb2_all[:],
                    in1=nm2[:, :, None].to_broadcast([P, NTT, MD2]),
                    op=OP.add)
                nc.vector.tensor_tensor(
                    out=t1a[:], in0=t1a[:],
                    in1=rstd2[:, :, None].to_broadcast([P, NTT, MD2]),
                    op=OP.mult)
                nc.vector.tensor_tensor(
                    out=t1a[:], in0=t1a[:],
                    in1=g2_sb[:, None, :].to_broadcast([P, NTT, MD2]),
                    op=OP.mult)
                nc.vector.tensor_tensor(
                    out=t1a[:], in0=t1a[:],
                    in1=be2_sb[:, None, :].to_broadcast([P, NTT, MD2]),
                    op=OP.add)
                nc.vector.tensor_scalar_max(t1a[:], t1a[:], 0.0)
                nc.vector.tensor_tensor(
                    out=t1a[:], in0=t1a[:],
                    in1=w3_sb[:, None, :].to_broadcast([P, NTT, MD2]),
                    op=OP.mult)
                base8 = sml.tile([P, NTT], f32, tag="base8")
                nc.vector.reduce_sum(base8[:], t1a[:], axis=mybir.AxisListType.X)
                nc.vector.tensor_tensor(
                    out=base8[:], in0=base8[:],
                    in1=b3_sb[:, 0:1].to_broadcast([P, NTT]), op=OP.add)
                imp1a = sml.tile([P, NTT], f32, tag="imp1a")
                nc.vector.tensor_scalar_add(imp1a[:], imp_all[:], 1.0)
                nc.vector.tensor_tensor(out=base8[:], in0=base8[:],
                                        in1=imp1a[:], op=OP.mult)
                nc.vector.tensor_scalar(base8[:], base8[:], MAX_W, MIN_W,
                                        op0=OP.min, op1=OP.max)
                nc.vector.tensor_tensor(out=res_sb[:], in0=base8[:],
                                        in1=maskf_sb[:], op=OP.mult)
                nc.sync.dma_start(out[:].rearrange("(t p) -> p t", p=P),
                                  res_sb[:])

    nc.compile()
    return nc


def _get_program():
    if "nc" not in _CACHE:
        _CACHE["nc"] = _build()
    return _CACHE["nc"]


def _prep_in_maps(inputs):
    import ml_dtypes
    bf = ml_dtypes.bfloat16

    hidden = np.asarray(inputs["hidden_states"], dtype=np.float32)
    token_ids = np.asarray(inputs["token_ids"], dtype=np.int32)
    mask = np.asarray(inputs["attention_mask"]).astype(bool)
    pos = np.asarray(inputs["pos_embed"], dtype=np.float32)
    in_proj_w = np.asarray(inputs["in_proj_w"], dtype=np.float32)
    in_proj_b = np.asarray(inputs["in_proj_b"], dtype=np.float32)
    out_w = np.asarray(inputs["out_w"], dtype=np.float32)
    out_b = np.asarray(inputs["out_b"], dtype=np.float32)
    w1 = np.asarray(inputs["w1"], dtype=np.float32)
    b1 = np.asarray(inputs["b1"], dtype=np.float32)
    g1 = np.asarray(inputs["g1"], dtype=np.float32)
    beta1 = np.asarray(inputs["beta1"], dtype=np.float32)
    w2 = np.asarray(inputs["w2"], dtype=np.float32)
    b2 = np.asarray(inputs["b2"], dtype=np.float32)
    g2 = np.asarray(inputs["g2"], dtype=np.float32)
    beta2 = np.asarray(inputs["beta2"], dtype=np.float32)
    w3 = np.asarray(inputs["w3"], dtype=np.float32)
    b3 = np.asarray(inputs["b3"], dtype=np.float32)
    table = np.asarray(inputs["importance_table"], dtype=np.float32)

    B, S_, H_ = hidden.shape
    assert (B, S_, H_) == (4, S, H), (B, S_, H_)

    x_full = hidden + pos                                      # [B, S, H]
    wqT = np.ascontiguousarray(in_proj_w[0:H].T.astype(bf))    # [H, H]
    wkT = np.ascontiguousarray(in_proj_w[H:2 * H].T.astype(bf))
    wvT = np.ascontiguousarray(in_proj_w[2 * H:3 * H].T.astype(bf))
    bq = in_proj_b[0:H] * INV_SQRT_HD      # fold score scale into q bias
    bk = in_proj_b[H:2 * H]
    bv = in_proj_b[2 * H:3 * H]
    # fold attention out-projection into the first meta layer:
    # w1 @ [x; att] + b1 == w1x @ x + (w1a @ out_w) @ ctx + (b1 + w1a @ out_b)
    w1x = w1[:, :H]
    w1a = w1[:, H:]
    w1eff = w1a @ out_w                                        # [MD, H]
    b1eff = b1 + w1a @ out_b
    w1cT = np.ascontiguousarray(
        np.concatenate([w1x, w1eff], axis=1).T.astype(bf))     # [2H, MD]
    w2T = np.ascontiguousarray(w2.T.astype(bf))                # [MD, MD2]

    def cmaj(v):   # [H] -> [128, H/128] partition-major (column dt holds v[dt*128+p])
        return np.ascontiguousarray(v.reshape(-1, P).T)

    def bcast(v):  # [F] -> [128, F]
        return np.ascontiguousarray(np.broadcast_to(v[None, :], (P, v.shape[0])))

    shared = {
        "wqT": wqT, "wkT": wkT, "wvT": wvT,
        "bq_c": cmaj(bq), "bk_c": cmaj(bk), "bv_b": bcast(bv),
        "w1cT": w1cT,
        "b1_cd": cmaj(b1eff), "g1_cd": cmaj(g1), "be1_cd": cmaj(beta1),
        "w2T": w2T, "b2_b": bcast(b2), "g2_b": bcast(g2), "be2_b": bcast(beta2),
        "w3_b": bcast(w3[0]), "b3_c": np.full((P, 1), b3[0], dtype=np.float32),
        "table": np.ascontiguousarray(table[:, None]),
    }

    in_maps = []
    for c in range(8):
        b = c // 2
        half = c % 2
        own = slice(half * SQ, (half + 1) * SQ)
        oth = slice((1 - half) * SQ, (2 - half) * SQ)
        xb = x_full[b].T                                       # [H, S] view
        # arrange so own half occupies columns [0, SQ)
        xT_arr = np.ascontiguousarray(
            np.concatenate([xb[:, own], xb[:, oth]], axis=1).astype(bf))
        kb = np.where(mask[b], 0.0, -1e9).astype(np.float32)
        kb_arr = np.concatenate([kb[own], kb[oth]])            # match column remap
        m = {
            "xT": xT_arr,
            "kbias": np.ascontiguousarray(kb_arr.reshape(-1, P).T),
            "maskf": np.ascontiguousarray(
                mask[b, own].astype(np.float32).reshape(-1, P).T),
            "tok": np.ascontiguousarray(token_ids[b, own][:, None]),
        }
        m.update(shared)
        in_maps.append(m)
    return in_maps


def _assemble(res):
    full = np.zeros((4, S), dtype=np.float32)
    for c in range(8):
        b = c // 2
        half = c % 2
        full[b, half * SQ:(half + 1) * SQ] = res.results[c]["out"]
    return full


def kernel(**inputs) -> np.ndarray:
    from concourse.bass_utils import run_bass_kernel_spmd
    in_maps = _prep_in_maps(inputs)
    nc = _get_program()
    res = run_bass_kernel_spmd(nc, in_maps, list(range(8)))
    return _assemble(res)


def run_traced(inputs, **kwargs):
    from concourse.bass_utils import run_bass_kernel_spmd
    in_maps = _prep_in_maps(inputs)
    nc = _get_program()
    return run_bass_kernel_spmd(nc, in_maps, list(range(8)), trace=True, **kwargs)


# revision 4
# speedup vs baseline: 1.3131x; 1.0846x over previous
"""Trainium2 Bass kernel for EnhancedMetaWeightNetwork.

Full (unsharded) inputs in, full output out. Internally: 8 NeuronCores,
core c handles batch b = c // 2 and query-row half c % 2 (1024 rows).
Attention K/V are computed per-core for the full sequence of the core's
batch (duplicated across the 2 cores sharing a batch; no collectives).

v3 layout strategy (bf16 storage, fp32 PSUM accumulation):
  - x = hidden + pos_embed precomputed on host; uploaded as bf16 x^T.
  - out_w folded into w1 on host (w1eff = w1[:, H:] @ out_w), so the
    attention out-projection disappears; h1 = w1x @ x + w1eff @ ctx.
  - everything SBUF-resident: no DRAM scratch.
  - fused per-head pipeline: K[0],Q[0] -> V (all) -> for each head:
    attention(h) interleaved with K/Q(h+1), so the scalar-engine exp
    stream hides under the PE matmul stream.
  - scoresT [key, query] per head in a 2-bank PSUM tile covering the
    full 1024 own queries; one exp activation per key-tile.
  - softmax denominator: pairwise ex adds on gpsimd + running
    accumulation on DVE, one ones-matmul broadcast, then
    reciprocal_approx_fast.
  - K/Q staging (PSUM -> bf16 SBUF + bias) on DVE to keep scalar free
    for exp; all constants packed into one DMA.
  - importance lookup via indirect DMA gather from the vocab table.
"""

import numpy as np

H = 1024
NH = 8
HD = 128           # head dim
S = 2048           # keys / full sequence
SQ = 1024          # own query rows per core
MD = 256           # meta dim
MD2 = 128
VOCAB = 32000
MIN_W, MAX_W = 0.1, 5.0
LN_EPS = 1e-5
P = 128
INV_SQRT_HD = 1.0 / np.sqrt(np.float32(HD))

# packed fp32 constant columns: [kbias 16 | maskf 8 | bq 8 | bk 8 | b1 2 |
#  g1 2 | be1 2 | b3 1 | w3 128 | b2 128 | g2 128 | be2 128 | bv 1024]
_CPK_SPANS = {}
_off = 0
for _name, _n in [("kbias", 16), ("maskf", 8), ("bq", 8), ("bk", 8),
                  ("b1", 2), ("g1", 2), ("be1", 2), ("b3", 1), ("w3", MD2),
                  ("b2", MD2), ("g2", MD2), ("be2", MD2), ("bv", H)]:
    _CPK_SPANS[_name] = (_off, _off + _n)
    _off += _n
NCPK = _off

_CACHE = {}


def _build():
    import concourse.bass as bass
    import concourse.mybir as mybir
    import concourse.tile as tile
    from concourse import bacc

    f32 = mybir.dt.float32
    bf16 = mybir.dt.bfloat16
    i32 = mybir.dt.int32
    OP = mybir.AluOpType
    ACT = mybir.ActivationFunctionType

    nc = bacc.Bacc("TRN2", target_bir_lowering=False, debug=False,
                   enable_asserts=False, num_devices=8)

    # ---------------- DRAM parameters ----------------
    dp = nc.declare_dram_parameter
    xT = dp("xT", [H, S], bf16, isOutput=False)           # (hidden+pos)[b].T, own half first
    wqT = dp("wqT", [H, H], bf16, isOutput=False)         # in_proj_w[0:H].T / sqrt(hd)
    wkT = dp("wkT", [H, H], bf16, isOutput=False)
    wvT = dp("wvT", [H, H], bf16, isOutput=False)
    w1cT = dp("w1cT", [2 * H, MD], bf16, isOutput=False)  # [w1x | w1a@out_w].T
    w2T = dp("w2T", [MD, MD2], bf16, isOutput=False)
    cpack = dp("cpack", [P, NCPK], f32, isOutput=False)
    tokc = dp("tokc", [P, SQ // P], i32, isOutput=False)
    table = dp("table", [VOCAB, 1], f32, isOutput=False)
    out = dp("out", [SQ], f32, isOutput=True)

    NKT = S // P          # 16 key tiles
    NC8 = H // P          # 8 feature chunks
    NTT = SQ // P         # 8 own token tiles
    NFT = MD // P         # 2 feature tiles of h1

    with tile.TileContext(nc) as tc:
        with tc.tile_pool(name="const", bufs=1) as cst, \
             tc.tile_pool(name="persist", bufs=1) as pa:

            # ---------------- constants (no DMA) ----------------
            ones_f = cst.tile([P, P], f32, tag="ones_f")
            nc.any.memset(ones_f[:], 1.0)
            ones_b = cst.tile([P, P], bf16, tag="ones_b")
            nc.vector.tensor_copy(ones_b[:], ones_f[:])
            eps_sb = cst.tile([P, 1], f32, tag="eps")
            nc.any.memset(eps_sb[:], LN_EPS)

            # ---- long-lived activations ----
            x_sb = pa.tile([P, NC8, S], bf16, tag="x")       # full x^T (own cols first)
            ctx_sb = pa.tile([P, NC8, SQ], bf16, tag="ctx")  # ctx^T (head-major)
            v_sb = pa.tile([P, NKT, H], bf16, tag="v")
            w1c_sb = pa.tile([P, 2 * NC8, MD], bf16, tag="w1c")
            h1p = pa.tile([P, NFT, SQ], bf16, tag="h1p")
            h1n = pa.tile([P, NFT, SQ], bf16, tag="h1n")
            res_sb = pa.tile([P, NTT], f32, tag="res")

            with tc.tile_pool(name="kqs", bufs=2) as kqs, \
                 tc.tile_pool(name="wst", bufs=2) as wst, \
                 tc.tile_pool(name="ps_kq", bufs=2, space="PSUM") as ps_kq:

                # ---- DMA issue order: head-0 weights, then x, then rest ----
                def load_w(src, h, tag):
                    t = wst.tile([P, NC8, P], bf16, tag=tag)
                    nc.sync.dma_start(t[:], src[:, h * P:(h + 1) * P]
                                      .rearrange("(c p) n -> p c n", p=P))
                    return t

                wk0 = load_w(wkT, 0, "wk")
                wq0 = load_w(wqT, 0, "wq")
                for c8 in range(NC8):
                    nc.sync.dma_start(x_sb[:, c8, :], xT[c8 * P:(c8 + 1) * P, :])

                cpk = cst.tile([P, NCPK], f32, tag="cpk")
                nc.sync.dma_start(cpk[:], cpack[:])

                def cslice(name):
                    lo, hi = _CPK_SPANS[name]
                    return cpk[:, lo:hi]

                kbias_sb = cslice("kbias")
                maskf_sb = cslice("maskf")
                bq_sb = cslice("bq")
                bk_sb = cslice("bk")
                b1_c = cslice("b1")
                g1_c = cslice("g1")
                be1_c = cslice("be1")
                b3_sb = cslice("b3")
                w3_sb = cslice("w3")
                b2_sb = cslice("b2")
                g2_sb = cslice("g2")
                be2_sb = cslice("be2")
                bv_sb = cslice("bv")

                tok_sb = cst.tile([P, NTT], i32, tag="tok")
                nc.sync.dma_start(tok_sb[:], tokc[:])
                w2_sb = cst.tile([P, NFT, MD2], bf16, tag="w2")
                nc.sync.dma_start(w2_sb[:],
                                  w2T[:].rearrange("(c p) n -> p c n", p=P))

                # importance gather (gpsimd queue; independent)
                imp_all = cst.tile([P, NTT], f32, tag="imp_all")
                for tt in range(NTT):
                    nc.gpsimd.indirect_dma_start(
                        out=imp_all[:, tt:tt + 1], out_offset=None, in_=table[:],
                        in_offset=bass.IndirectOffsetOnAxis(
                            ap=tok_sb[:, tt:tt + 1], axis=0))

                # ---- per-head K/Q projection + staging (DVE) ----
                def emit_k(h, wk_sb):
                    k_h = kqs.tile([P, S], bf16, tag="k_h")
                    for sb in range(S // 512):
                        psk = ps_kq.tile([P, 512], mybir.dt.float32, tag="kq",
                                         name="psk")
                        for c8 in range(NC8):
                            nc.tensor.matmul(psk[:], lhsT=wk_sb[:, c8, :],
                                             rhs=x_sb[:, c8, sb * 512:(sb + 1) * 512],
                                             start=(c8 == 0), stop=(c8 == NC8 - 1))
                        nc.vector.tensor_tensor(
                            out=k_h[:, sb * 512:(sb + 1) * 512], in0=psk[:],
                            in1=bk_sb[:, h:h + 1].to_broadcast([P, 512]),
                            op=OP.add)
                    return k_h

                def emit_q(h, wq_sb):
                    q_h = kqs.tile([P, SQ], bf16, tag="q_h")
                    for qb in range(SQ // 512):
                        psq = ps_kq.tile([P, 512], mybir.dt.float32, tag="kq",
                                         name="psq")
                        for c8 in range(NC8):
                            nc.tensor.matmul(psq[:], lhsT=wq_sb[:, c8, :],
                                             rhs=x_sb[:, c8, qb * 512:(qb + 1) * 512],
                                             start=(c8 == 0), stop=(c8 == NC8 - 1))
                        nc.vector.tensor_tensor(
                            out=q_h[:, qb * 512:(qb + 1) * 512], in0=psq[:],
                            in1=bq_sb[:, h:h + 1].to_broadcast([P, 512]),
                            op=OP.add)
                    return q_h

                k_cur = emit_k(0, wk0)
                q_cur = emit_q(0, wq0)

                # ---- V projection (full sequence) ----
                with tc.tile_pool(name="wvp", bufs=1) as wvp, \
                     tc.tile_pool(name="ps_v", bufs=3, space="PSUM") as ps_v:
                    wv_sb = wvp.tile([P, NC8, H], bf16, tag="wv")
                    for db in range(H // 512):
                        nc.sync.dma_start(
                            wv_sb[:, :, db * 512:(db + 1) * 512],
                            wvT[:, db * 512:(db + 1) * 512]
                            .rearrange("(c p) n -> p c n", p=P))
                    nc.sync.dma_start(w1c_sb[:],
                                      w1cT[:].rearrange("(c p) n -> p c n", p=P))

                    for tt in range(NKT):
                        psv = ps_v.tile([P, 1024], mybir.dt.float32, tag="v2")
                        for c8 in range(NC8):
                            lhsT = x_sb[:, c8, tt * P:(tt + 1) * P]
                            for db in range(H // 512):
                                nc.tensor.matmul(psv[:, db * 512:(db + 1) * 512],
                                                 lhsT=lhsT,
                                                 rhs=wv_sb[:, c8, db * 512:(db + 1) * 512],
                                                 start=(c8 == 0), stop=(c8 == NC8 - 1))
                        nc.vector.tensor_tensor(out=v_sb[:, tt, :], in0=psv[:],
                                                in1=bv_sb[:], op=OP.add)

                # ---- fused attention + next-head K/Q pipeline ----
                with tc.tile_pool(name="exps", bufs=4) as exps, \
                     tc.tile_pool(name="trp", bufs=3) as trp, \
                     tc.tile_pool(name="trd", bufs=2) as trd, \
                     tc.tile_pool(name="rcps", bufs=2) as rcps, \
                     tc.tile_pool(name="ps_sc", bufs=2, space="PSUM") as ps_sc, \
                     tc.tile_pool(name="ps_cps", bufs=1, space="PSUM") as ps_cps:
                    for h in range(NH):
                        if h + 1 < NH:
                            wk_nx = load_w(wkT, h + 1, "wk")
                            wq_nx = load_w(wqT, h + 1, "wq")

                        cps = ps_cps.tile([P, 1024], mybir.dt.float32, tag="cps")
                        pair = None
                        acc = None
                        for kt in range(NKT):
                            sc = ps_sc.tile([P, 1024], mybir.dt.float32, tag="sc")
                            for qb in range(SQ // 512):
                                nc.tensor.matmul(
                                    sc[:, qb * 512:(qb + 1) * 512],
                                    lhsT=k_cur[:, kt * P:(kt + 1) * P],
                                    rhs=q_cur[:, qb * 512:(qb + 1) * 512],
                                    start=True, stop=True)
                            ex = exps.tile([P, 1024], bf16, tag="ex")
                            nc.scalar.activation(ex[:], sc[:], ACT.Exp,
                                                 bias=kbias_sb[:, kt:kt + 1],
                                                 scale=1.0)
                            for qb in range(SQ // 512):
                                nc.tensor.matmul(
                                    cps[:, qb * 512:(qb + 1) * 512],
                                    lhsT=v_sb[:, kt, h * P:(h + 1) * P],
                                    rhs=ex[:, qb * 512:(qb + 1) * 512],
                                    start=(kt == 0), stop=(kt == NKT - 1))
                            if kt % 2 == 0:
                                ex_even = ex
                            else:
                                # pairwise add on gpsimd (SBUF-only engine)
                                pair = trp.tile([P, 1024], bf16, tag="pair")
                                nc.gpsimd.tensor_tensor(out=pair[:], in0=ex_even[:],
                                                        in1=ex[:], op=OP.add)
                                # running accumulation on DVE
                                if kt == 1:
                                    acc = pair
                                else:
                                    nacc = trd.tile([P, 1024], bf16, tag="acc")
                                    nc.vector.tensor_tensor(out=nacc[:], in0=acc[:],
                                                            in1=pair[:], op=OP.add)
                                    acc = nacc

                        # denominator broadcast via ones-matmul, then approx
                        # reciprocal and context scale (emitted after K(h+1)
                        # so the PE never waits on the adder tree)
                        def finish_head(h, cps, acc):
                            rcb = rcps.tile([P, 1024], f32, tag="rcb")
                            for qb in range(SQ // 512):
                                dnf = ps_kq.tile([P, 512], mybir.dt.float32,
                                                 tag="kq", name="dnf")
                                nc.tensor.matmul(dnf[:], lhsT=ones_b[:],
                                                 rhs=acc[:, qb * 512:(qb + 1) * 512],
                                                 start=True, stop=True)
                                with nc.allow_low_precision(reason="softmax rcp"):
                                    nc.vector.reciprocal_approx_fast(
                                        out=rcb[:, qb * 512:(qb + 1) * 512],
                                        in_=dnf[:])
                            nc.vector.tensor_tensor(out=ctx_sb[:, h, :], in0=cps[:],
                                                    in1=rcb[:], op=OP.mult)

                        if h + 1 < NH:
                            k_cur = emit_k(h + 1, wk_nx)
                            finish_head(h, cps, acc)
                            q_cur = emit_q(h + 1, wq_nx)
                        else:
                            finish_head(h, cps, acc)

            # ---------- meta MLP ----------
            with tc.tile_pool(name="mw", bufs=1) as mw, \
                 tc.tile_pool(name="msml", bufs=2) as sml, \
                 tc.tile_pool(name="ps_m", bufs=6, space="PSUM") as ps2:
                # ---- h1 in feature-major: h1preT [256, SQ] ----
                for ft in range(NFT):
                    for qb in range(SQ // 512):
                        psf_t = ps2.tile([P, 512], mybir.dt.float32,
                                         tag="mm512", name="psf")
                        for j in range(2 * NC8):
                            if j < NC8:
                                rhs = x_sb[:, j, qb * 512:(qb + 1) * 512]
                            else:
                                rhs = ctx_sb[:, j - NC8, qb * 512:(qb + 1) * 512]
                            nc.tensor.matmul(
                                psf_t[:],
                                lhsT=w1c_sb[:, j, ft * P:(ft + 1) * P],
                                rhs=rhs,
                                start=(j == 0), stop=(j == 2 * NC8 - 1))
                        nc.scalar.activation(
                            h1p[:, ft, qb * 512:(qb + 1) * 512], psf_t[:],
                            ACT.Identity, bias=b1_c[:, ft:ft + 1], scale=1.0)
                for qb in range(SQ // 512):
                    qsl = slice(qb * 512, (qb + 1) * 512)
                    h1sqs = []
                    for ft in range(NFT):
                        h1sq = sml.tile([P, 512], bf16, tag=f"h1sq{ft}")
                        nc.vector.tensor_tensor(out=h1sq[:], in0=h1p[:, ft, qsl],
                                                in1=h1p[:, ft, qsl], op=OP.mult)
                        h1sqs.append(h1sq)
                    psA = ps2.tile([P, 512], mybir.dt.float32, tag="mm512",
                                   name="psA")
                    psB = ps2.tile([P, 512], mybir.dt.float32, tag="mm512",
                                   name="psB")
                    for ft in range(NFT):
                        nc.tensor.matmul(psA[:], lhsT=ones_b[:],
                                         rhs=h1p[:, ft, qsl],
                                         start=(ft == 0), stop=(ft == NFT - 1))
                    for ft in range(NFT):
                        nc.tensor.matmul(psB[:], lhsT=ones_b[:],
                                         rhs=h1sqs[ft][:],
                                         start=(ft == 0), stop=(ft == NFT - 1))
                    nmean = sml.tile([P, 512], f32, tag="nmean")
                    ex2m = sml.tile([P, 512], f32, tag="ex2m")
                    m2r = sml.tile([P, 512], f32, tag="m2r")
                    nc.vector.tensor_scalar_mul(nmean[:], psA[:], -1.0 / MD)
                    nc.vector.tensor_scalar_mul(ex2m[:], psB[:], 1.0 / MD)
                    nc.vector.tensor_tensor(out=m2r[:], in0=nmean[:],
                                            in1=nmean[:], op=OP.mult)
                    nc.vector.tensor_tensor(out=ex2m[:], in0=ex2m[:],
                                            in1=m2r[:], op=OP.subtract)
                    # rstd = exp(-0.5 * ln(var + eps)) on ACT (fast path)
                    nc.scalar.activation(ex2m[:], ex2m[:], ACT.Ln,
                                         bias=eps_sb[:, 0:1], scale=1.0)
                    rstd = sml.tile([P, 512], bf16, tag="rstd")
                    with nc.allow_low_precision(reason="bf16 layernorm scale"):
                        nc.scalar.activation(rstd[:], ex2m[:], ACT.Exp,
                                             bias=0.0, scale=-0.5)
                    for ft in range(NFT):
                        h1c = sml.tile([P, 512], bf16, tag=f"h1c{ft}")
                        nc.vector.tensor_tensor(out=h1c[:], in0=h1p[:, ft, qsl],
                                                in1=nmean[:], op=OP.add)
                        nc.vector.tensor_tensor(out=h1c[:], in0=h1c[:],
                                                in1=rstd[:], op=OP.mult)
                        nc.scalar.activation(h1n[:, ft, qsl], h1c[:],
                                             ACT.Relu, bias=be1_c[:, ft:ft + 1],
                                             scale=g1_c[:, ft:ft + 1])

                # ---- h2 + LN2/final, two token groups to overlap the tail ----
                F2 = float(MD2)
                NG = 2
                GT = NTT // NG      # 4 token tiles per group
                for g in range(NG):
                    hb2 = mw.tile([P, GT, MD2], f32, tag="hb2", bufs=2)
                    for ti in range(GT):
                        tt = g * GT + ti
                        ph2_t = ps2.tile([P, 512], mybir.dt.float32, tag="mm512",
                                         name="ph2")
                        ph2 = ph2_t[:, :MD2]
                        for ft in range(NFT):
                            nc.tensor.matmul(ph2,
                                             lhsT=h1n[:, ft, tt * P:(tt + 1) * P],
                                             rhs=w2_sb[:, ft, :],
                                             start=(ft == 0), stop=(ft == NFT - 1))
                        nc.vector.scalar_tensor_tensor(out=hb2[:, ti, :], in0=ph2,
                                                       scalar=1.0, in1=b2_sb[:],
                                                       op0=OP.mult, op1=OP.add)
                    gsl = slice(g * GT, (g + 1) * GT)
                    sums2 = sml.tile([P, GT], f32, tag="sums2")
                    nc.vector.reduce_sum(sums2[:], hb2[:],
                                         axis=mybir.AxisListType.X)
                    msq = sml.tile([P, GT, MD2], f32, tag="msq")
                    ssq2 = sml.tile([P, GT], f32, tag="ssq2")
                    nc.vector.tensor_tensor(out=msq[:], in0=hb2[:],
                                            in1=hb2[:], op=OP.mult)
                    nc.vector.reduce_sum(ssq2[:], msq[:], axis=mybir.AxisListType.X)
                    nm2 = sml.tile([P, GT], f32, tag="nm2")
                    nc.vector.tensor_scalar_mul(nm2[:], sums2[:], -1.0 / F2)
                    ex22 = sml.tile([P, GT], f32, tag="ex22")
                    nc.vector.tensor_scalar_mul(ex22[:], ssq2[:], 1.0 / F2)
                    mm2 = sml.tile([P, GT], f32, tag="mm2")
                    nc.vector.tensor_tensor(out=mm2[:], in0=nm2[:], in1=nm2[:],
                                            op=OP.mult)
                    var2 = sml.tile([P, GT], f32, tag="var2")
                    nc.vector.tensor_tensor(out=var2[:], in0=ex22[:], in1=mm2[:],
                                            op=OP.subtract)
                    std2 = sml.tile([P, GT], f32, tag="std2")
                    nc.scalar.activation(std2[:], var2[:], ACT.Sqrt,
                                         bias=eps_sb[:, 0:1], scale=1.0)
                    rstd2 = sml.tile([P, GT], f32, tag="rstd2")
                    nc.vector.reciprocal(rstd2[:], std2[:])
                    t1a = sml.tile([P, GT, MD2], f32, tag="t1a")
                    nc.vector.tensor_tensor(
                        out=t1a[:], in0=hb2[:],
                        in1=nm2[:, :, None].to_broadcast([P, GT, MD2]),
                        op=OP.add)
                    nc.vector.tensor_tensor(
                        out=t1a[:], in0=t1a[:],
                        in1=rstd2[:, :, None].to_broadcast([P, GT, MD2]),
                        op=OP.mult)
                    nc.vector.tensor_tensor(
                        out=t1a[:], in0=t1a[:],
                        in1=g2_sb[:, None, :].to_broadcast([P, GT, MD2]),
                        op=OP.mult)
                    nc.vector.tensor_tensor(
                        out=t1a[:], in0=t1a[:],
                        in1=be2_sb[:, None, :].to_broadcast([P, GT, MD2]),
                        op=OP.add)
                    nc.vector.tensor_scalar_max(t1a[:], t1a[:], 0.0)
                    nc.vector.tensor_tensor(
                        out=t1a[:], in0=t1a[:],
                        in1=w3_sb[:, None, :].to_broadcast([P, GT, MD2]),
                        op=OP.mult)
                    base8 = sml.tile([P, GT], f32, tag="base8")
                    nc.vector.reduce_sum(base8[:], t1a[:], axis=mybir.AxisListType.X)
                    nc.vector.tensor_tensor(
                        out=base8[:], in0=base8[:],
                        in1=b3_sb[:, 0:1].to_broadcast([P, GT]), op=OP.add)
                    imp1a = sml.tile([P, GT], f32, tag="imp1a")
                    nc.vector.tensor_scalar_add(imp1a[:], imp_all[:, gsl], 1.0)
                    nc.vector.tensor_tensor(out=base8[:], in0=base8[:],
                                            in1=imp1a[:], op=OP.mult)
                    nc.vector.tensor_scalar(base8[:], base8[:], MAX_W, MIN_W,
                                            op0=OP.min, op1=OP.max)
                    nc.vector.tensor_tensor(out=res_sb[:, gsl], in0=base8[:],
                                            in1=maskf_sb[:, gsl], op=OP.mult)
                nc.sync.dma_start(out[:].rearrange("(t p) -> p t", p=P),
                                  res_sb[:])

    nc.compile()
    return nc


def _get_program():
    if "nc" not in _CACHE:
        _CACHE["nc"] = _build()
    return _CACHE["nc"]


def _prep_in_maps(inputs):
    import ml_dtypes
    bf = ml_dtypes.bfloat16

    hidden = np.asarray(inputs["hidden_states"], dtype=np.float32)
    token_ids = np.asarray(inputs["token_ids"], dtype=np.int32)
    mask = np.asarray(inputs["attention_mask"]).astype(bool)
    pos = np.asarray(inputs["pos_embed"], dtype=np.float32)
    in_proj_w = np.asarray(inputs["in_proj_w"], dtype=np.float32)
    in_proj_b = np.asarray(inputs["in_proj_b"], dtype=np.float32)
    out_w = np.asarray(inputs["out_w"], dtype=np.float32)
    out_b = np.asarray(inputs["out_b"], dtype=np.float32)
    w1 = np.asarray(inputs["w1"], dtype=np.float32)
    b1 = np.asarray(inputs["b1"], dtype=np.float32)
    g1 = np.asarray(inputs["g1"], dtype=np.float32)
    beta1 = np.asarray(inputs["beta1"], dtype=np.float32)
    w2 = np.asarray(inputs["w2"], dtype=np.float32)
    b2 = np.asarray(inputs["b2"], dtype=np.float32)
    g2 = np.asarray(inputs["g2"], dtype=np.float32)
    beta2 = np.asarray(inputs["beta2"], dtype=np.float32)
    w3 = np.asarray(inputs["w3"], dtype=np.float32)
    b3 = np.asarray(inputs["b3"], dtype=np.float32)
    table = np.asarray(inputs["importance_table"], dtype=np.float32)

    B, S_, H_ = hidden.shape
    assert (B, S_, H_) == (4, S, H), (B, S_, H_)

    x_full = hidden + pos                                      # [B, S, H]
    # fold 1/sqrt(hd) into the q projection (weights and bias)
    wqT_ = np.ascontiguousarray(
        (in_proj_w[0:H].T * INV_SQRT_HD).astype(bf))           # [H, H]
    wkT_ = np.ascontiguousarray(in_proj_w[H:2 * H].T.astype(bf))
    wvT_ = np.ascontiguousarray(in_proj_w[2 * H:3 * H].T.astype(bf))
    bq = in_proj_b[0:H] * INV_SQRT_HD
    bk = in_proj_b[H:2 * H]
    bv = in_proj_b[2 * H:3 * H]
    # fold attention out-projection into the first meta layer:
    # w1 @ [x; att] + b1 == w1x @ x + (w1a @ out_w) @ ctx + (b1 + w1a @ out_b)
    w1x = w1[:, :H]
    w1a = w1[:, H:]
    w1eff = w1a @ out_w                                        # [MD, H]
    b1eff = b1 + w1a @ out_b
    w1cT = np.ascontiguousarray(
        np.concatenate([w1x, w1eff], axis=1).T.astype(bf))     # [2H, MD]
    w2T_ = np.ascontiguousarray(w2.T.astype(bf))               # [MD, MD2]

    def cmaj(v):   # [F] -> [128, F/128] partition-major
        return np.ascontiguousarray(v.reshape(-1, P).T)

    def bcast(v):  # [F] -> [128, F]
        return np.ascontiguousarray(np.broadcast_to(v[None, :], (P, v.shape[0])))

    def pack_consts(kb_arr, maskf_arr):
        cp = np.zeros((P, NCPK), dtype=np.float32)
        def put(name, arr):
            lo, hi = _CPK_SPANS[name]
            cp[:, lo:hi] = arr
        put("kbias", cmaj(kb_arr))
        put("maskf", maskf_arr)
        put("bq", cmaj(bq))
        put("bk", cmaj(bk))
        put("b1", cmaj(b1eff))
        put("g1", cmaj(g1))
        put("be1", cmaj(beta1))
        put("b3", np.full((P, 1), b3[0], dtype=np.float32))
        put("w3", bcast(w3[0]))
        put("b2", bcast(b2))
        put("g2", bcast(g2))
        put("be2", bcast(beta2))
        put("bv", bcast(bv))
        return cp

    shared = {
        "wqT": wqT_, "wkT": wkT_, "wvT": wvT_,
        "w1cT": w1cT, "w2T": w2T_,
        "table": np.ascontiguousarray(table[:, None]),
    }

    in_maps = []
    for c in range(8):
        b = c // 2
        half = c % 2
        own = slice(half * SQ, (half + 1) * SQ)
        oth = slice((1 - half) * SQ, (2 - half) * SQ)
        xb = x_full[b].T                                       # [H, S] view
        # arrange so own half occupies columns [0, SQ)
        xT_arr = np.ascontiguousarray(
            np.concatenate([xb[:, own], xb[:, oth]], axis=1).astype(bf))
        kb = np.where(mask[b], 0.0, -1e9).astype(np.float32)
        kb_arr = np.concatenate([kb[own], kb[oth]])            # match column remap
        maskf_arr = np.ascontiguousarray(
            mask[b, own].astype(np.float32).reshape(-1, P).T)
        m = {
            "xT": xT_arr,
            "cpack": pack_consts(kb_arr, maskf_arr),
            "tokc": np.ascontiguousarray(token_ids[b, own].reshape(-1, P).T),
        }
        m.update(shared)
        in_maps.append(m)
    return in_maps


def _assemble(res):
    full = np.zeros((4, S), dtype=np.float32)
    for c in range(8):
        b = c // 2
        half = c % 2
        full[b, half * SQ:(half + 1) * SQ] = res.results[c]["out"]
    return full


def kernel(**inputs) -> np.ndarray:
    from concourse.bass_utils import run_bass_kernel_spmd
    in_maps = _prep_in_maps(inputs)
    nc = _get_program()
    res = run_bass_kernel_spmd(nc, in_maps, list(range(8)))
    return _assemble(res)


def run_traced(inputs, **kwargs):
    from concourse.bass_utils import run_bass_kernel_spmd
    in_maps = _prep_in_maps(inputs)
    nc = _get_program()
    return run_bass_kernel_spmd(nc, in_maps, list(range(8)), trace=True, **kwargs)


# revision 9
# speedup vs baseline: 1.4624x; 1.1137x over previous
"""Trainium2 Bass kernel for EnhancedMetaWeightNetwork.

Full (unsharded) inputs in, full output out. Internally: 8 NeuronCores,
core c handles batch b = c // 2 and query-row half c % 2 (1024 rows).
Attention K/V are computed per-core for the full sequence of the core's
batch (duplicated across the 2 cores sharing a batch; no collectives).

v3 layout strategy (bf16 storage, fp32 PSUM accumulation):
  - x = hidden + pos_embed precomputed on host; uploaded as bf16 x^T.
  - out_w folded into w1 on host (w1eff = w1[:, H:] @ out_w), so the
    attention out-projection disappears; h1 = w1x @ x + w1eff @ ctx.
  - everything SBUF-resident: no DRAM scratch.
  - fused per-head pipeline: K[0],Q[0] -> V (all) -> for each head:
    attention(h) interleaved with K/Q(h+1), so the scalar-engine exp
    stream hides under the PE matmul stream.
  - scoresT [key, query] per head in a 2-bank PSUM tile covering the
    full 1024 own queries; one exp activation per key-tile.
  - softmax denominator: pairwise ex adds on gpsimd + running
    accumulation on DVE, one ones-matmul broadcast, then
    reciprocal_approx_fast.
  - K/Q staging (PSUM -> bf16 SBUF + bias) on DVE to keep scalar free
    for exp; all constants packed into one DMA.
  - importance lookup via indirect DMA gather from the vocab table.
"""

import numpy as np

H = 1024
NH = 8
HD = 128           # head dim
S = 2048           # keys / full sequence
SQ = 1024          # own query rows per core
MD = 256           # meta dim
MD2 = 128
VOCAB = 32000
MIN_W, MAX_W = 0.1, 5.0
LN_EPS = 1e-5
P = 128
INV_SQRT_HD = 1.0 / np.sqrt(np.float32(HD))

# packed fp32 constant columns: [kbias 16 | maskf 8 | bq 8 | bk 8 | b1 2 |
#  g1 2 | be1 2 | b3 1 | w3 128 | b2 128 | g2 128 | be2 128 | bv 1024]
_CPK_SPANS = {}
_off = 0
for _name, _n in [("kbias", 16), ("maskf", 8), ("bq", 8), ("bk", 8),
                  ("b1", 2), ("g1", 2), ("be1", 2), ("b3", 1), ("w3", MD2),
                  ("b2", MD2), ("g2", MD2), ("be2", MD2), ("bv", H)]:
    _CPK_SPANS[_name] = (_off, _off + _n)
    _off += _n
NCPK = _off

_CACHE = {}


def _build():
    import concourse.bass as bass
    import concourse.mybir as mybir
    import concourse.tile as tile
    from concourse import bacc

    f32 = mybir.dt.float32
    bf16 = mybir.dt.bfloat16
    i32 = mybir.dt.int32
    OP = mybir.AluOpType
    ACT = mybir.ActivationFunctionType

    nc = bacc.Bacc("TRN2", target_bir_lowering=False, debug=False,
                   enable_asserts=False, num_devices=8)

    # ---------------- DRAM parameters ----------------
    dp = nc.declare_dram_parameter
    xT = dp("xT", [H, S], bf16, isOutput=False)           # (hidden+pos)[b].T, own half first
    wqT = dp("wqT", [H, H], bf16, isOutput=False)         # in_proj_w[0:H].T / sqrt(hd)
    wkT = dp("wkT", [H, H], bf16, isOutput=False)
    wvT = dp("wvT", [H, H], bf16, isOutput=False)
    w1cT = dp("w1cT", [2 * H, MD], bf16, isOutput=False)  # [w1x | w1a@out_w].T
    w2T = dp("w2T", [MD, MD2], bf16, isOutput=False)
    cpack = dp("cpack", [P, NCPK], f32, isOutput=False)
    tokc = dp("tokc", [P, SQ // P], i32, isOutput=False)
    table = dp("table", [VOCAB, 1], f32, isOutput=False)
    out = dp("out", [SQ], f32, isOutput=True)

    NKT = S // P          # 16 key tiles
    NC8 = H // P          # 8 feature chunks
    NTT = SQ // P         # 8 own token tiles
    NFT = MD // P         # 2 feature tiles of h1

    with tile.TileContext(nc) as tc:
        with tc.tile_pool(name="const", bufs=1) as cst, \
             tc.tile_pool(name="persist", bufs=1) as pa:

            # ---------------- constants (no DMA) ----------------
            ones_f = cst.tile([P, P], f32, tag="ones_f")
            nc.any.memset(ones_f[:], 1.0)
            ones_b = cst.tile([P, P], bf16, tag="ones_b")
            nc.vector.tensor_copy(ones_b[:], ones_f[:])
            eps_sb = cst.tile([P, 1], f32, tag="eps")
            nc.any.memset(eps_sb[:], LN_EPS)

            # ---- long-lived activations ----
            x_sb = pa.tile([P, NC8, S], bf16, tag="x")       # full x^T (own cols first)
            ctx_sb = pa.tile([P, NC8, SQ], bf16, tag="ctx")  # ctx^T (head-major)
            v_sb = pa.tile([P, NKT, H], bf16, tag="v")
            w1c_sb = pa.tile([P, 2 * NC8, MD], bf16, tag="w1c")
            h1p = pa.tile([P, NFT, SQ], bf16, tag="h1p")
            h1n = pa.tile([P, NFT, SQ], bf16, tag="h1n")
            res_sb = pa.tile([P, NTT], f32, tag="res")

            with tc.tile_pool(name="kqs", bufs=2) as kqs, \
                 tc.tile_pool(name="wst", bufs=2) as wst, \
                 tc.tile_pool(name="ps_kq", bufs=2, space="PSUM") as ps_kq:

                # ---- DMA issue order: head-0 weights, then x, then rest ----
                def load_w(src, h, tag):
                    t = wst.tile([P, NC8, P], bf16, tag=tag)
                    nc.sync.dma_start(t[:], src[:, h * P:(h + 1) * P]
                                      .rearrange("(c p) n -> p c n", p=P))
                    return t

                wk0 = load_w(wkT, 0, "wk")
                wq0 = load_w(wqT, 0, "wq")
                # load x by column blocks so K(0) can start after the first
                for cb in range(S // 512):
                    nc.sync.dma_start(
                        x_sb[:, :, cb * 512:(cb + 1) * 512],
                        xT[:, cb * 512:(cb + 1) * 512]
                        .rearrange("(c p) n -> p c n", p=P))

                cpk = cst.tile([P, NCPK], f32, tag="cpk")
                nc.sync.dma_start(cpk[:], cpack[:])

                def cslice(name):
                    lo, hi = _CPK_SPANS[name]
                    return cpk[:, lo:hi]

                kbias_sb = cslice("kbias")
                maskf_sb = cslice("maskf")
                bq_sb = cslice("bq")
                bk_sb = cslice("bk")
                b1_c = cslice("b1")
                g1_c = cslice("g1")
                be1_c = cslice("be1")
                b3_sb = cslice("b3")
                w3_sb = cslice("w3")
                b2_sb = cslice("b2")
                g2_sb = cslice("g2")
                be2_sb = cslice("be2")
                bv_sb = cslice("bv")

                tok_sb = cst.tile([P, NTT], i32, tag="tok")
                nc.sync.dma_start(tok_sb[:], tokc[:])
                w2_sb = cst.tile([P, NFT, MD2], bf16, tag="w2")
                nc.sync.dma_start(w2_sb[:],
                                  w2T[:].rearrange("(c p) n -> p c n", p=P))

                # importance gather (gpsimd queue; independent)
                imp_all = cst.tile([P, NTT], f32, tag="imp_all")
                for tt in range(NTT):
                    nc.gpsimd.indirect_dma_start(
                        out=imp_all[:, tt:tt + 1], out_offset=None, in_=table[:],
                        in_offset=bass.IndirectOffsetOnAxis(
                            ap=tok_sb[:, tt:tt + 1], axis=0))

                # ---- per-head K/Q projection + staging (DVE) ----
                def emit_k(h, wk_sb):
                    k_h = kqs.tile([P, S], bf16, tag="k_h")
                    for sb in range(S // 512):
                        psk = ps_kq.tile([P, 512], mybir.dt.float32, tag="kq",
                                         name="psk")
                        for c8 in range(NC8):
                            nc.tensor.matmul(psk[:], lhsT=wk_sb[:, c8, :],
                                             rhs=x_sb[:, c8, sb * 512:(sb + 1) * 512],
                                             start=(c8 == 0), stop=(c8 == NC8 - 1))
                        nc.vector.tensor_tensor(
                            out=k_h[:, sb * 512:(sb + 1) * 512], in0=psk[:],
                            in1=bk_sb[:, h:h + 1].to_broadcast([P, 512]),
                            op=OP.add)
                    return k_h

                def emit_q(h, wq_sb):
                    q_h = kqs.tile([P, SQ], bf16, tag="q_h")
                    for qb in range(SQ // 512):
                        psq = ps_kq.tile([P, 512], mybir.dt.float32, tag="kq",
                                         name="psq")
                        for c8 in range(NC8):
                            nc.tensor.matmul(psq[:], lhsT=wq_sb[:, c8, :],
                                             rhs=x_sb[:, c8, qb * 512:(qb + 1) * 512],
                                             start=(c8 == 0), stop=(c8 == NC8 - 1))
                        nc.vector.tensor_tensor(
                            out=q_h[:, qb * 512:(qb + 1) * 512], in0=psq[:],
                            in1=bq_sb[:, h:h + 1].to_broadcast([P, 512]),
                            op=OP.add)
                    return q_h

                k_cur = emit_k(0, wk0)
                q_cur = emit_q(0, wq0)

                # ---- V projection (full sequence) ----
                with tc.tile_pool(name="wvp", bufs=1) as wvp, \
                     tc.tile_pool(name="ps_v", bufs=3, space="PSUM") as ps_v:
                    wv_sb = wvp.tile([P, NC8, H], bf16, tag="wv")
                    for db in range(H // 512):
                        nc.sync.dma_start(
                            wv_sb[:, :, db * 512:(db + 1) * 512],
                            wvT[:, db * 512:(db + 1) * 512]
                            .rearrange("(c p) n -> p c n", p=P))
                    nc.sync.dma_start(w1c_sb[:],
                                      w1cT[:].rearrange("(c p) n -> p c n", p=P))

                    for tt in range(NKT):
                        psv = ps_v.tile([P, 1024], mybir.dt.float32, tag="v2")
                        for c8 in range(NC8):
                            lhsT = x_sb[:, c8, tt * P:(tt + 1) * P]
                            for db in range(H // 512):
                                nc.tensor.matmul(psv[:, db * 512:(db + 1) * 512],
                                                 lhsT=lhsT,
                                                 rhs=wv_sb[:, c8, db * 512:(db + 1) * 512],
                                                 start=(c8 == 0), stop=(c8 == NC8 - 1))
                        nc.vector.tensor_tensor(out=v_sb[:, tt, :], in0=psv[:],
                                                in1=bv_sb[:], op=OP.add)

                # ---- fused attention + next-head K/Q pipeline ----
                with tc.tile_pool(name="exps", bufs=4) as exps, \
                     tc.tile_pool(name="trp", bufs=3) as trp, \
                     tc.tile_pool(name="trd", bufs=2) as trd, \
                     tc.tile_pool(name="rcps", bufs=2) as rcps, \
                     tc.tile_pool(name="ps_sc", bufs=2, space="PSUM") as ps_sc, \
                     tc.tile_pool(name="ps_cps", bufs=1, space="PSUM") as ps_cps:
                    for h in range(NH):
                        if h + 1 < NH:
                            wk_nx = load_w(wkT, h + 1, "wk")
                            wq_nx = load_w(wqT, h + 1, "wq")

                        cps = ps_cps.tile([P, 1024], mybir.dt.float32, tag="cps")
                        pair = None
                        acc = None
                        for kt in range(NKT):
                            sc = ps_sc.tile([P, 1024], mybir.dt.float32, tag="sc")
                            for qb in range(SQ // 512):
                                nc.tensor.matmul(
                                    sc[:, qb * 512:(qb + 1) * 512],
                                    lhsT=k_cur[:, kt * P:(kt + 1) * P],
                                    rhs=q_cur[:, qb * 512:(qb + 1) * 512],
                                    start=True, stop=True)
                            ex = exps.tile([P, 1024], bf16, tag="ex")
                            nc.scalar.activation(ex[:], sc[:], ACT.Exp,
                                                 bias=kbias_sb[:, kt:kt + 1],
                                                 scale=1.0)
                            for qb in range(SQ // 512):
                                nc.tensor.matmul(
                                    cps[:, qb * 512:(qb + 1) * 512],
                                    lhsT=v_sb[:, kt, h * P:(h + 1) * P],
                                    rhs=ex[:, qb * 512:(qb + 1) * 512],
                                    start=(kt == 0), stop=(kt == NKT - 1))
                            if kt % 2 == 0:
                                ex_even = ex
                            else:
                                # pairwise add + running accumulation on DVE
                                pair = trp.tile([P, 1024], bf16, tag="pair")
                                nc.vector.tensor_tensor(out=pair[:], in0=ex_even[:],
                                                        in1=ex[:], op=OP.add)
                                if kt == 1:
                                    acc = pair
                                else:
                                    nacc = trd.tile([P, 1024], bf16, tag="acc")
                                    nc.vector.tensor_tensor(out=nacc[:], in0=acc[:],
                                                            in1=pair[:], op=OP.add)
                                    acc = nacc

                        # denominator broadcast via ones-matmul, then approx
                        # reciprocal and context scale (emitted after K(h+1)
                        # so the PE never waits on the adder tree)
                        def finish_head(h, cps, acc):
                            rcb = rcps.tile([P, 1024], f32, tag="rcb")
                            for qb in range(SQ // 512):
                                dnf = ps_kq.tile([P, 512], mybir.dt.float32,
                                                 tag="kq", name="dnf")
                                nc.tensor.matmul(dnf[:], lhsT=ones_b[:],
                                                 rhs=acc[:, qb * 512:(qb + 1) * 512],
                                                 start=True, stop=True)
                                with nc.allow_low_precision(reason="softmax rcp"):
                                    nc.vector.reciprocal_approx_fast(
                                        out=rcb[:, qb * 512:(qb + 1) * 512],
                                        in_=dnf[:])
                            nc.vector.tensor_tensor(out=ctx_sb[:, h, :], in0=cps[:],
                                                    in1=rcb[:], op=OP.mult)

                        if h + 1 < NH:
                            k_cur = emit_k(h + 1, wk_nx)
                            finish_head(h, cps, acc)
                            q_cur = emit_q(h + 1, wq_nx)
                        else:
                            finish_head(h, cps, acc)

            # ---------- meta MLP ----------
            with tc.tile_pool(name="mw", bufs=1) as mw, \
                 tc.tile_pool(name="msml", bufs=2) as sml, \
                 tc.tile_pool(name="ps_m", bufs=6, space="PSUM") as ps2:
                # ---- h1 in feature-major: h1preT [256, SQ] (qb-outer so
                # LN1(qb0) overlaps the qb1 matmuls on the PE) ----
                for qb in range(SQ // 512):
                    qsl = slice(qb * 512, (qb + 1) * 512)
                    for ft in range(NFT):
                        psf_t = ps2.tile([P, 512], mybir.dt.float32,
                                         tag="mm512", name="psf")
                        for j in range(2 * NC8):
                            if j < NC8:
                                rhs = x_sb[:, j, qsl]
                            else:
                                rhs = ctx_sb[:, j - NC8, qsl]
                            nc.tensor.matmul(
                                psf_t[:],
                                lhsT=w1c_sb[:, j, ft * P:(ft + 1) * P],
                                rhs=rhs,
                                start=(j == 0), stop=(j == 2 * NC8 - 1))
                        nc.scalar.activation(
                            h1p[:, ft, qsl], psf_t[:],
                            ACT.Identity, bias=b1_c[:, ft:ft + 1], scale=1.0)
                    h1sqs = []
                    for ft in range(NFT):
                        h1sq = sml.tile([P, 512], bf16, tag=f"h1sq{ft}")
                        nc.vector.tensor_tensor(out=h1sq[:], in0=h1p[:, ft, qsl],
                                                in1=h1p[:, ft, qsl], op=OP.mult)
                        h1sqs.append(h1sq)
                    psA = ps2.tile([P, 512], mybir.dt.float32, tag="mm512",
                                   name="psA")
                    psB = ps2.tile([P, 512], mybir.dt.float32, tag="mm512",
                                   name="psB")
                    for ft in range(NFT):
                        nc.tensor.matmul(psA[:], lhsT=ones_b[:],
                                         rhs=h1p[:, ft, qsl],
                                         start=(ft == 0), stop=(ft == NFT - 1))
                    for ft in range(NFT):
                        nc.tensor.matmul(psB[:], lhsT=ones_b[:],
                                         rhs=h1sqs[ft][:],
                                         start=(ft == 0), stop=(ft == NFT - 1))
                    nmean = sml.tile([P, 512], f32, tag="nmean")
                    ex2m = sml.tile([P, 512], f32, tag="ex2m")
                    m2r = sml.tile([P, 512], f32, tag="m2r")
                    nc.vector.tensor_scalar_mul(nmean[:], psA[:], -1.0 / MD)
                    nc.vector.tensor_scalar_mul(ex2m[:], psB[:], 1.0 / MD)
                    nc.vector.tensor_tensor(out=m2r[:], in0=nmean[:],
                                            in1=nmean[:], op=OP.mult)
                    nc.vector.tensor_tensor(out=ex2m[:], in0=ex2m[:],
                                            in1=m2r[:], op=OP.subtract)
                    # rstd = exp(-0.5 * ln(var + eps)) on ACT (fast path)
                    nc.scalar.activation(ex2m[:], ex2m[:], ACT.Ln,
                                         bias=eps_sb[:, 0:1], scale=1.0)
                    rstd = sml.tile([P, 512], bf16, tag="rstd")
                    with nc.allow_low_precision(reason="bf16 layernorm scale"):
                        nc.scalar.activation(rstd[:], ex2m[:], ACT.Exp,
                                             bias=0.0, scale=-0.5)
                    for ft in range(NFT):
                        h1c = sml.tile([P, 512], bf16, tag=f"h1c{ft}")
                        nc.vector.tensor_tensor(out=h1c[:], in0=h1p[:, ft, qsl],
                                                in1=nmean[:], op=OP.add)
                        nc.vector.tensor_tensor(out=h1c[:], in0=h1c[:],
                                                in1=rstd[:], op=OP.mult)
                        nc.scalar.activation(h1n[:, ft, qsl], h1c[:],
                                             ACT.Relu, bias=be1_c[:, ft:ft + 1],
                                             scale=g1_c[:, ft:ft + 1])

                # ---- h2 + LN2/final, two token groups to overlap the tail ----
                F2 = float(MD2)
                NG = 2
                GT = NTT // NG      # 4 token tiles per group
                for g in range(NG):
                    hb2 = mw.tile([P, GT, MD2], f32, tag="hb2", bufs=2)
                    for ti in range(GT):
                        tt = g * GT + ti
                        ph2_t = ps2.tile([P, 512], mybir.dt.float32, tag="mm512",
                                         name="ph2")
                        ph2 = ph2_t[:, :MD2]
                        for ft in range(NFT):
                            nc.tensor.matmul(ph2,
                                             lhsT=h1n[:, ft, tt * P:(tt + 1) * P],
                                             rhs=w2_sb[:, ft, :],
                                             start=(ft == 0), stop=(ft == NFT - 1))
                        nc.vector.scalar_tensor_tensor(out=hb2[:, ti, :], in0=ph2,
                                                       scalar=1.0, in1=b2_sb[:],
                                                       op0=OP.mult, op1=OP.add)
                    gsl = slice(g * GT, (g + 1) * GT)
                    sums2 = sml.tile([P, GT], f32, tag="sums2")
                    nc.vector.reduce_sum(sums2[:], hb2[:],
                                         axis=mybir.AxisListType.X)
                    msq = sml.tile([P, GT, MD2], f32, tag="msq")
                    ssq2 = sml.tile([P, GT], f32, tag="ssq2")
                    nc.vector.tensor_tensor(out=msq[:], in0=hb2[:],
                                            in1=hb2[:], op=OP.mult)
                    nc.vector.reduce_sum(ssq2[:], msq[:], axis=mybir.AxisListType.X)
                    nm2 = sml.tile([P, GT], f32, tag="nm2")
                    nc.vector.tensor_scalar_mul(nm2[:], sums2[:], -1.0 / F2)
                    ex22 = sml.tile([P, GT], f32, tag="ex22")
                    nc.vector.tensor_scalar_mul(ex22[:], ssq2[:], 1.0 / F2)
                    mm2 = sml.tile([P, GT], f32, tag="mm2")
                    nc.vector.tensor_tensor(out=mm2[:], in0=nm2[:], in1=nm2[:],
                                            op=OP.mult)
                    var2 = sml.tile([P, GT], f32, tag="var2")
                    nc.vector.tensor_tensor(out=var2[:], in0=ex22[:], in1=mm2[:],
                                            op=OP.subtract)
                    std2 = sml.tile([P, GT], f32, tag="std2")
                    nc.scalar.activation(std2[:], var2[:], ACT.Sqrt,
                                         bias=eps_sb[:, 0:1], scale=1.0)
                    rstd2 = sml.tile([P, GT], f32, tag="rstd2")
                    nc.vector.reciprocal(rstd2[:], std2[:])
                    t1a = sml.tile([P, GT, MD2], f32, tag="t1a")
                    nc.vector.tensor_tensor(
                        out=t1a[:], in0=hb2[:],
                        in1=nm2[:, :, None].to_broadcast([P, GT, MD2]),
                        op=OP.add)
                    nc.vector.tensor_tensor(
                        out=t1a[:], in0=t1a[:],
                        in1=rstd2[:, :, None].to_broadcast([P, GT, MD2]),
                        op=OP.mult)
                    nc.vector.tensor_tensor(
                        out=t1a[:], in0=t1a[:],
                        in1=g2_sb[:, None, :].to_broadcast([P, GT, MD2]),
                        op=OP.mult)
                    nc.vector.tensor_tensor(
                        out=t1a[:], in0=t1a[:],
                        in1=be2_sb[:, None, :].to_broadcast([P, GT, MD2]),
                        op=OP.add)
                    nc.vector.tensor_scalar_max(t1a[:], t1a[:], 0.0)
                    nc.vector.tensor_tensor(
                        out=t1a[:], in0=t1a[:],
                        in1=w3_sb[:, None, :].to_broadcast([P, GT, MD2]),
                        op=OP.mult)
                    base8 = sml.tile([P, GT], f32, tag="base8")
                    nc.vector.reduce_sum(base8[:], t1a[:], axis=mybir.AxisListType.X)
                    nc.vector.tensor_tensor(
                        out=base8[:], in0=base8[:],
                        in1=b3_sb[:, 0:1].to_broadcast([P, GT]), op=OP.add)
                    imp1a = sml.tile([P, GT], f32, tag="imp1a")
                    nc.vector.tensor_scalar_add(imp1a[:], imp_all[:, gsl], 1.0)
                    nc.vector.tensor_tensor(out=base8[:], in0=base8[:],
                                            in1=imp1a[:], op=OP.mult)
                    nc.vector.tensor_scalar(base8[:], base8[:], MAX_W, MIN_W,
                                            op0=OP.min, op1=OP.max)
                    nc.vector.tensor_tensor(out=res_sb[:, gsl], in0=base8[:],
                                            in1=maskf_sb[:, gsl], op=OP.mult)
                nc.sync.dma_start(out[:].rearrange("(t p) -> p t", p=P),
                                  res_sb[:])

    nc.compile()
    return nc


def _get_program():
    if "nc" not in _CACHE:
        _CACHE["nc"] = _build()
    return _CACHE["nc"]


def _prep_in_maps(inputs):
    import ml_dtypes
    bf = ml_dtypes.bfloat16

    hidden = np.asarray(inputs["hidden_states"], dtype=np.float32)
    token_ids = np.asarray(inputs["token_ids"], dtype=np.int32)
    mask = np.asarray(inputs["attention_mask"]).astype(bool)
    pos = np.asarray(inputs["pos_embed"], dtype=np.float32)
    in_proj_w = np.asarray(inputs["in_proj_w"], dtype=np.float32)
    in_proj_b = np.asarray(inputs["in_proj_b"], dtype=np.float32)
    out_w = np.asarray(inputs["out_w"], dtype=np.float32)
    out_b = np.asarray(inputs["out_b"], dtype=np.float32)
    w1 = np.asarray(inputs["w1"], dtype=np.float32)
    b1 = np.asarray(inputs["b1"], dtype=np.float32)
    g1 = np.asarray(inputs["g1"], dtype=np.float32)
    beta1 = np.asarray(inputs["beta1"], dtype=np.float32)
    w2 = np.asarray(inputs["w2"], dtype=np.float32)
    b2 = np.asarray(inputs["b2"], dtype=np.float32)
    g2 = np.asarray(inputs["g2"], dtype=np.float32)
    beta2 = np.asarray(inputs["beta2"], dtype=np.float32)
    w3 = np.asarray(inputs["w3"], dtype=np.float32)
    b3 = np.asarray(inputs["b3"], dtype=np.float32)
    table = np.asarray(inputs["importance_table"], dtype=np.float32)

    B, S_, H_ = hidden.shape
    assert (B, S_, H_) == (4, S, H), (B, S_, H_)

    x_full = hidden + pos                                      # [B, S, H]
    # fold 1/sqrt(hd) into the q projection (weights and bias)
    wqT_ = np.ascontiguousarray(
        (in_proj_w[0:H].T * INV_SQRT_HD).astype(bf))           # [H, H]
    wkT_ = np.ascontiguousarray(in_proj_w[H:2 * H].T.astype(bf))
    wvT_ = np.ascontiguousarray(in_proj_w[2 * H:3 * H].T.astype(bf))
    bq = in_proj_b[0:H] * INV_SQRT_HD
    bk = in_proj_b[H:2 * H]
    bv = in_proj_b[2 * H:3 * H]
    # fold attention out-projection into the first meta layer:
    # w1 @ [x; att] + b1 == w1x @ x + (w1a @ out_w) @ ctx + (b1 + w1a @ out_b)
    w1x = w1[:, :H]
    w1a = w1[:, H:]
    w1eff = w1a @ out_w                                        # [MD, H]
    b1eff = b1 + w1a @ out_b
    w1cT = np.ascontiguousarray(
        np.concatenate([w1x, w1eff], axis=1).T.astype(bf))     # [2H, MD]
    w2T_ = np.ascontiguousarray(w2.T.astype(bf))               # [MD, MD2]

    def cmaj(v):   # [F] -> [128, F/128] partition-major
        return np.ascontiguousarray(v.reshape(-1, P).T)

    def bcast(v):  # [F] -> [128, F]
        return np.ascontiguousarray(np.broadcast_to(v[None, :], (P, v.shape[0])))

    def pack_consts(kb_arr, maskf_arr):
        cp = np.zeros((P, NCPK), dtype=np.float32)
        def put(name, arr):
            lo, hi = _CPK_SPANS[name]
            cp[:, lo:hi] = arr
        put("kbias", cmaj(kb_arr))
        put("maskf", maskf_arr)
        put("bq", cmaj(bq))
        put("bk", cmaj(bk))
        put("b1", cmaj(b1eff))
        put("g1", cmaj(g1))
        put("be1", cmaj(beta1))
        put("b3", np.full((P, 1), b3[0], dtype=np.float32))
        put("w3", bcast(w3[0]))
        put("b2", bcast(b2))
        put("g2", bcast(g2))
        put("be2", bcast(beta2))
        put("bv", bcast(bv))
        return cp

    shared = {
        "wqT": wqT_, "wkT": wkT_, "wvT": wvT_,
        "w1cT": w1cT, "w2T": w2T_,
        "table": np.ascontiguousarray(table[:, None]),
    }

    in_maps = []
    for c in range(8):
        b = c // 2
        half = c % 2
        own = slice(half * SQ, (half + 1) * SQ)
        oth = slice((1 - half) * SQ, (2 - half) * SQ)
        xb = x_full[b].T                                       # [H, S] view
        # arrange so own half occupies columns [0, SQ)
        xT_arr = np.ascontiguousarray(
            np.concatenate([xb[:, own], xb[:, oth]], axis=1).astype(bf))
        kb = np.where(mask[b], 0.0, -1e9).astype(np.float32)
        kb_arr = np.concatenate([kb[own], kb[oth]])            # match column remap
        maskf_arr = np.ascontiguousarray(
            mask[b, own].astype(np.float32).reshape(-1, P).T)
        m = {
            "xT": xT_arr,
            "cpack": pack_consts(kb_arr, maskf_arr),
            "tokc": np.ascontiguousarray(token_ids[b, own].reshape(-1, P).T),
        }
        m.update(shared)
        in_maps.append(m)
    return in_maps


def _assemble(res):
    full = np.zeros((4, S), dtype=np.float32)
    for c in range(8):
        b = c // 2
        half = c % 2
        full[b, half * SQ:(half + 1) * SQ] = res.results[c]["out"]
    return full


def kernel(**inputs) -> np.ndarray:
    from concourse.bass_utils import run_bass_kernel_spmd
    in_maps = _prep_in_maps(inputs)
    nc = _get_program()
    res = run_bass_kernel_spmd(nc, in_maps, list(range(8)))
    return _assemble(res)


def run_traced(inputs, **kwargs):
    from concourse.bass_utils import run_bass_kernel_spmd
    in_maps = _prep_in_maps(inputs)
    nc = _get_program()
    return run_bass_kernel_spmd(nc, in_maps, list(range(8)), trace=True, **kwargs)
